# revision 54
# baseline (speedup 1.0000x reference)
"""Gated Linear Attention on 8 Trainium2 NeuronCores.

Sharding: one (batch, head) pair per core (B=2 x H=4 = 8 cores). Each core
computes its head's full pipeline and emits a partial [N, D] output (bf16);
the host sums the 4 head partials per batch in f32.

v6 design (89.6us -> 83.3us vs v4):
  * All heavy matmuls in bf16 (1 PE cycle/row vs 4 for fp32); PSUM accums f32.
  * Per-chunk LOCAL decay (no global cumsum carry chain): within chunk c,
    b = L^T g'' (local inclusive cumsum). q~=q*exp(-b/16), k~=k*exp(+b/16);
    cross-chunk state rescaled once per chunk by the per-feature factor
    f = exp(-b_last/16):  W_c = diag(f) (W_{c-1} + k~^T v).
  * Weight blob split into bank1 (gate|z, 384 cols) and bank0 (q|k|v, 512
    cols) DRAM tensors, quartered across the three DMA rings so bank1 +
    x chunk 0 land first: P1(0) starts ~12us instead of ~21us; wb0
    quarters pace P0(0) behind it. Input load is bandwidth-floored at
    ~300GB/s aggregate - only the ORDER is tunable.
  * Prologue projects chunks 0 AND 1's bank1 during the load window; QK
    evictions are hoisted one iteration early (right after P0(c+1)
    closes) so T(c) never waits on the DVE.
  * Per-chunk PE order: Bmm, T/TR2 transposes FIRST, then P1/P0(c+1),
    A, OT before FINmm(c-1) (OT's at_m is ready early; the swap shifts
    each chunk's SQ/SSQ/R tail ~1us earlier, relaxing the cross-chunk
    R->GATE->TR2->og chain that feeds FIN).
  * PSUM: sml (bT|at|ssq) double-buffered - load-bearing; tq|tr2 packed
    into ONE bank ([128,512] tile, 4 transposes per iteration).
  * Consts packed into one small DMA (Lb|idb); L (f32) derived on-chip.
  * Warmup = 8 matmuls (ends right as the first wb1 quarter lands).
  * All mid-loop stores on the sync ring (a store issue on the scalar
    ring delays the ACT queue); last chunk's store column-split with
    per-half cast->issue on vector/scalar + sync/scalar rings.
  * ACT table discipline: exp+ln resolve to the combined table by blanking
    the exp-only/ln-only sets for the load-insertion pass (ids still index
    the real act_info.json). Silu via reciprocal_approx_fast. 2 loads total.
  * RMS r deferred and folded into the silu gate; gateT eviction on ACT;
    bf16 I/O; contiguous 2KB+ DMA rows. GpSimd/Pool is used ONLY for DMA
    issue - any Pool compute op on the chunk path costs multi-us dispatch.
"""

import os
from contextlib import ExitStack

import numpy as np
import ml_dtypes

import concourse.bass as bass
import concourse.tile as tile
from concourse import bacc, mybir
from concourse.tile_rust import add_dep_helper
from concourse.bass_utils import run_bass_kernel_spmd

F32 = mybir.dt.float32
BF16 = mybir.dt.bfloat16
AF = mybir.ActivationFunctionType
ALU = mybir.AluOpType

B, N, D, H = 2, 1024, 1024, 4
KD, VD, DK, DV = 512, 1024, 128, 256
C = 128                    # chunk length (= token partitions)
NCH = N // C               # 8 chunks
NK = D // 128              # 8 contraction tiles
GLN = 16.0
EPS = 1e-5
E48 = float(np.exp(48.0).astype(np.float32))

# module-level stash so test.py can grab profiling results
LAST_RESULTS = None

_BLANK_TABLES = ("exp_and_others", "natural_log", "exp_and_friends")
_tables_patched = False


def _patch_act_tables():
    """Steer the ACT-table-load chooser toward natural_log_exp_and_others so
    exp+ln never alternate table loads. Only the (name -> funcs) map used by
    the load-insertion pass and CoreSim is filtered; emitted act_func_set_ids
    still index the real act_info.json, so walrus/hardware see valid sets."""
    global _tables_patched
    if _tables_patched:
        return
    _tables_patched = True
    from concourse import hw_specs, bass_interp
    orig = hw_specs.get_activation_tables

    def patched(arch):
        tabs = dict(orig(arch))
        for name in _BLANK_TABLES:
            if name in tabs:
                tabs[name] = set()
        return tabs

    bacc.get_activation_tables = patched
    bass_interp.get_activation_tables = patched



def _emit_kernel(ctx: ExitStack, tc: "tile.TileContext", ap: dict):
    nc = tc.nc

    # Chain all PE instructions in program order (PE executes in-order; this
    # keeps the Tile scheduler from reordering matmuls within a PSUM bank,
    # which would break has_written clear ordering).
    pe_prev = [None]

    def mm(*args, **kw):
        inst = nc.tensor.matmul(*args, **kw)
        if kw.get("skip_group_check") or kw.get("start") in (False, None):
            # keep explicit order only for matmuls that join open psum groups
            if pe_prev[0] is not None:
                add_dep_helper(inst.ins, pe_prev[0], sync=False,
                               reason="pe-order")
        pe_prev[0] = inst.ins
        return inst

    def tr_(out, in_, ident):
        inst = nc.tensor.transpose(out, in_, ident)
        if pe_prev[0] is not None:
            add_dep_helper(inst.ins, pe_prev[0], sync=False, reason="pe-order")
        pe_prev[0] = inst.ins
        return inst

    xT, wb1d, wb0d, woutT = ap["xT"], ap["wb1"], ap["wb0"], ap["woutT"]
    bgk2, cb = ap["bgk2"], ap["cb"]
    out = ap["out"]

    consts = ctx.enter_context(tc.tile_pool(name="consts", bufs=1))
    wpool = ctx.enter_context(tc.tile_pool(name="wpool", bufs=1))
    work = ctx.enter_context(tc.tile_pool(name="work", bufs=3))
    wst = ctx.enter_context(tc.tile_pool(name="wst", bufs=2))
    outp = ctx.enter_context(tc.tile_pool(name="outp", bufs=3))
    ppool = ctx.enter_context(tc.tile_pool(name="ppool", bufs=2, space="PSUM"))
    ptr = ctx.enter_context(tc.tile_pool(name="ptr", bufs=2, space="PSUM"))
    pbf = ctx.enter_context(tc.tile_pool(name="pbf", bufs=1, space="PSUM"))
    pao = ctx.enter_context(tc.tile_pool(name="pao", bufs=1, space="PSUM"))

    # PE clock warmup source: FIRST vector instruction so the warmup matmuls
    # can launch while the DMA rings fill. The tensor engine needs ~3us of
    # continuous execution to reach max frequency.
    warm = consts.tile([128, 512], BF16)
    nc.vector.memset(warm[:], 0.0)

    # ---- DMA schedule: need-ordered across the three ~110GB/s rings.
    # Ring layout (issue order = per-engine program order):
    #   gpsimd: x0 | cb(Lb|idb) | x1 | x2 | x3 | x6
    #   sync:   wb1[0:4] | wb0[0:2] | wb0[4:6] | wout0 | x4
    #   scalar: bgk2 | wb1[4:8] | wb0[2:4] | wb0[6:8] | wout1 | x5 | x7
    # bank1 (gate|z) + x0 land ~10.5us -> P1(0) starts; wb0 quarters pace
    # P0(0) to ~15.5us; later x chunks have chunk-pipeline slack.
    xsb = wpool.tile([128, N, NK], BF16)
    wb1_sb = wpool.tile([128, NK, 384], BF16)
    wb0_sb = wpool.tile([128, NK, 512], BF16)
    wout_sb = wpool.tile([128, 2, D], BF16)
    cb_sb = consts.tile([128, 256], BF16)
    bg_sb = consts.tile([1, 128], BF16)

    nc.gpsimd.dma_start(out=xsb[:, 0:C, :], in_=xT[:, 0:C, :])
    nc.scalar.dma_start(out=bg_sb[:], in_=bgk2[:])
    nc.sync.dma_start(out=wb1_sb[:, 0:2, :], in_=wb1d[:, 0:2, :])
    nc.scalar.dma_start(out=wb1_sb[:, 2:4, :], in_=wb1d[:, 2:4, :])
    nc.sync.dma_start(out=wb1_sb[:, 4:6, :], in_=wb1d[:, 4:6, :])
    nc.scalar.dma_start(out=wb1_sb[:, 6:8, :], in_=wb1d[:, 6:8, :])
    # x1 ahead of cb: P1(1) needs x1 ~2us before anything reads Lb/idb
    nc.gpsimd.dma_start(out=xsb[:, C:2 * C, :], in_=xT[:, C:2 * C, :])
    nc.sync.dma_start(out=wb0_sb[:, 0:2, :], in_=wb0d[:, 0:2, :])
    nc.scalar.dma_start(out=wb0_sb[:, 2:4, :], in_=wb0d[:, 2:4, :])
    nc.gpsimd.dma_start(out=cb_sb[:], in_=cb[:])
    nc.sync.dma_start(out=wb0_sb[:, 4:6, :], in_=wb0d[:, 4:6, :])
    nc.scalar.dma_start(out=wb0_sb[:, 6:8, :], in_=wb0d[:, 6:8, :])
    nc.gpsimd.dma_start(out=xsb[:, 2 * C:3 * C, :], in_=xT[:, 2 * C:3 * C, :])
    nc.sync.dma_start(out=wout_sb[:, 0, :], in_=woutT[0])
    nc.scalar.dma_start(out=wout_sb[:, 1, :], in_=woutT[1])
    nc.gpsimd.dma_start(out=xsb[:, 3 * C:4 * C, :], in_=xT[:, 3 * C:4 * C, :])
    nc.sync.dma_start(out=xsb[:, 4 * C:5 * C, :], in_=xT[:, 4 * C:5 * C, :])
    nc.scalar.dma_start(out=xsb[:, 5 * C:6 * C, :], in_=xT[:, 5 * C:6 * C, :])
    nc.gpsimd.dma_start(out=xsb[:, 6 * C:7 * C, :], in_=xT[:, 6 * C:7 * C, :])
    nc.scalar.dma_start(out=xsb[:, 7 * C:8 * C, :], in_=xT[:, 7 * C:8 * C, :])

    # remaining consts on vector (idle during the load phase)
    ones_row = consts.tile([1, 128], BF16)
    nc.vector.memset(ones_row[:], 1.0)
    ones_col = consts.tile([128, 1], BF16)
    nc.vector.memset(ones_col[:], 1.0)
    eps_sb = consts.tile([128, 1], F32)
    nc.vector.memset(eps_sb[:], EPS)
    Lb_sb = cb_sb[:, 0:128]               # L[s,t]=1 iff s<=t (triu), bf16
    idb_sb = cb_sb[:, 128:256]            # identity, bf16
    L_sb = consts.tile([128, 128], F32)
    nc.vector.tensor_copy(L_sb[:], cb_sb[:, 0:128])

    # PE clock warmup: dummy matmuls while the first DMAs land.
    wps = pao.tile([128, 512], F32, tag="big")
    for i in range(8):
        mm(wps[:], lhsT=warm[:, 0:128], rhs=warm[:],
           start=(i == 0), stop=(i == 7))

    # ---- main loop ---------------------------------------------------------
    # proj psum [128,1024]: bank0 {q 0:128 | k 128:256 | v 256:512}
    # bank1 {gate 512:768 | z 768:896 | b_loc 896:1024}. bank1 (and its bias
    # close) is emitted BEFORE bank0 so softplus overlaps the qkv matmuls.
    # b (token-major) and bT (feature-major) are both produced directly by
    # matmuls against the triangular mask (b = L^T g, bT = g^T L).
    # The ENTIRE output path (silu gate via reciprocal_approx_fast - no
    # activation-table switch - RMS scale, final projection, store) is inlined
    # per chunk, one chunk behind the front of the pipeline, so outputs
    # stream to HBM throughout the loop and no drain phase remains.

    def P1(c):
        proj = ppool.tile([128, 1024], F32, tag="proj")
        tok = slice(c * C, (c + 1) * C)
        for k in range(NK):
            mm(proj[:, 512:896], lhsT=xsb[:, tok, k], rhs=wb1_sb[:, k, :],
               start=(k == 0), stop=False)
        bias_mm = mm(proj[:, 768:896], lhsT=ones_row[:], rhs=bg_sb[:],
                     start=False, stop=True)
        # softplus part a: e1 = exp(-z)
        e1 = work.tile([128, 128], F32, tag="e1")
        i = nc.scalar.activation(e1[:], proj[:, 768:896], AF.Exp, scale=-1.0)
        add_dep_helper(i.ins, bias_mm.ins, sync=False, reason="z after close")
        return proj, e1

    def SPb(c, e1):
        u1 = work.tile([128, 128], F32, tag="u1")
        nc.vector.tensor_scalar(u1[:], e1[:], 1.0, E48, ALU.add, ALU.min)
        return u1

    def SPc(c, u1):
        g_c = work.tile([128, 128], BF16, tag="g")
        nc.scalar.activation(g_c[:], u1[:], AF.Ln)
        return g_c

    def P0(c, proj):
        tok = slice(c * C, (c + 1) * C)
        for k in range(NK):
            mm(proj[:, 0:512], lhsT=xsb[:, tok, k], rhs=wb0_sb[:, k, :],
               start=(k == 0), stop=(k == NK - 1))

    def Bmm(c, proj, g_c):
        bmm = mm(proj[:, 896:1024], lhsT=Lb_sb, rhs=g_c[:],
                 start=False, stop=False, skip_group_check=True)
        sml = ptr.tile([128, 512], F32, tag="sml")   # bT | at | ssq
        mm(sml[:, 0:128], lhsT=g_c[:], rhs=Lb_sb, start=True, stop=True)
        return sml, bmm

    def Ex(c, proj, sml, bmm):
        En_tok = work.tile([128, 128], BF16, tag="Ent")
        i = nc.scalar.activation(En_tok[:], proj[:, 896:1024], AF.Exp,
                                 scale=1.0 / GLN)
        add_dep_helper(i.ins, bmm.ins, sync=False, reason="b after b-mm")
        ET = work.tile([128, 128], BF16, tag="ET")
        nc.scalar.activation(ET[:], sml[:, 0:128], AF.Exp, scale=-1.0 / GLN)
        EnT = work.tile([128, 128], BF16, tag="EnT")
        nc.scalar.activation(EnT[:], sml[:, 0:128], AF.Exp, scale=1.0 / GLN)
        f_vec = work.tile([128, 1], F32, tag="f")
        nc.scalar.activation(f_vec[:], sml[:, 127:128], AF.Exp, scale=-1.0 / GLN)
        # silu ingredient: eg = exp(-ug) straight from psum
        eg = work.tile([128, DV], F32, tag="eg")
        nc.scalar.activation(eg[:], proj[:, 512:768], AF.Exp, scale=-1.0)
        return En_tok, ET, EnT, f_vec, eg

    def QK(c, proj):
        qk_sb = work.tile([128, 256], BF16, tag="qk")
        nc.vector.tensor_copy(qk_sb[:], proj[:, 0:256])
        v_tm = work.tile([128, DV], BF16, tag="v")
        nc.scalar.copy(v_tm[:], proj[:, 256:512])
        ug = work.tile([128, DV], F32, tag="ug")
        nc.scalar.copy(ug[:], proj[:, 512:768])
        return qk_sb, v_tm, ug

    def T(c, qk_sb):
        # one psum bank holds this iteration's 4 transposes: qT|kT|gateT(c-1)
        tq = pbf.tile([128, 512], BF16, tag="tqk")
        tr_(tq[:, 0:128], qk_sb[:, 0:128], idb_sb)
        tr_(tq[:, 128:256], qk_sb[:, 128:256], idb_sb)
        return tq

    def M(c, tq, qk_sb, En_tok, ET, EnT):
        qtT = work.tile([128, 128], BF16, tag="qtT")
        nc.vector.tensor_mul(qtT[:], tq[:, 0:128], ET[:])
        ktT = work.tile([128, 128], BF16, tag="ktT")
        nc.vector.tensor_mul(ktT[:], tq[:, 128:256], EnT[:])
        kt_tm = work.tile([128, 128], BF16, tag="kt")
        nc.vector.tensor_mul(kt_tm[:], qk_sb[:, 128:256], En_tok[:])
        return qtT, ktT, kt_tm

    def SILU(c, eg):
        # silu: rf = 1/(1+eg); emitted after at_m so the critical DVE ops
        # (qtT/ktT/kt_tm/og/at) run first. Pool engine is NOT used for
        # per-chunk ops: its dispatch latency is multi-us.
        dg = work.tile([128, DV], F32, tag="dg")
        nc.vector.tensor_scalar_add(dg[:], eg[:], 1.0)
        rf = work.tile([128, DV], F32, tag="rf")
        nc.vector.reciprocal_approx_fast(rf[:], dg[:])
        return rf

    def A(c, sml, qtT, ktT):
        mm(sml[:, 128:256], lhsT=ktT[:], rhs=qtT[:], start=True, stop=True)

    def AM(c, sml):
        at_m = work.tile([128, 128], BF16, tag="atm")
        nc.vector.tensor_mul(at_m[:], sml[:, 128:256], L_sb[:])
        return at_m

    def OT(c, at_m, qtT, v_tm):
        big = pao.tile([128, 512], F32, tag="big")
        ot = big[:, 0:256]
        if c > 0:
            w_prev = state["w_prev_for_o"]
            mm(ot[:, 0:128], lhsT=w_prev[:, 0:128], rhs=qtT[:],
               start=True, stop=False)
            mm(ot[:, 128:256], lhsT=w_prev[:, 128:256], rhs=qtT[:],
               start=False, stop=False, skip_group_check=True)
            mm(ot[:, 0:128], lhsT=v_tm[:, 0:128], rhs=at_m[:],
               start=False, stop=False, skip_group_check=True)
        else:
            mm(ot[:, 0:128], lhsT=v_tm[:, 0:128], rhs=at_m[:],
               start=True, stop=False)
        mm(ot[:, 128:256], lhsT=v_tm[:, 128:256], rhs=at_m[:],
           start=False, stop=False, skip_group_check=True)
        return big

    def ST(c, big, kt_tm, v_tm, f_vec):
        if c == NCH - 1:
            return   # final state is never consumed
        st = big[:, 256:512]
        mm(st[:], lhsT=kt_tm[:], rhs=v_tm[:], start=True, stop=False,
           skip_group_check=True)
        if c > 0:
            mm(st[:], lhsT=idb_sb, rhs=state["w_prev"][:], start=False,
               stop=False, skip_group_check=True)
        w_new = wst.tile([128, DV], BF16, tag="w")
        nc.vector.tensor_scalar(w_new[:], st[:], f_vec[:], None, ALU.mult)
        state["w_prev"] = w_new

    def SQ(c, big):
        sq = work.tile([128, DV], BF16, tag="sq")
        nc.scalar.square(sq[:], big[:, 0:256])
        return sq

    def SSQ(c, sq, sml):
        ssq = sml[:, 256:257]
        mm(ssq, lhsT=sq[:, 0:128], rhs=ones_col[:],
           start=True, stop=False, skip_group_check=True)
        mm(ssq, lhsT=sq[:, 128:256], rhs=ones_col[:],
           start=False, stop=False, skip_group_check=True)
        return ssq

    def R(c, ssq):
        s_c = work.tile([128, 1], F32, tag="s")
        nc.scalar.activation(s_c[:], ssq, AF.Ln, scale=1.0 / DV, bias=eps_sb[:])
        r_c = work.tile([128, 1], F32, tag="r")
        nc.scalar.activation(r_c[:], s_c[:], AF.Exp, scale=-0.5)
        return r_c

    def GATE(c, ug, rf, r_c):
        # gate*r = (ug*r) * sigmoid(ug), sigmoid via fast reciprocal
        gate_tm = work.tile([128, DV], BF16, tag="gate")
        nc.vector.scalar_tensor_tensor(gate_tm[:], ug[:], r_c[:], rf[:],
                                       ALU.mult, ALU.mult)
        return gate_tm

    def TR2(c, gate_tm, tq):
        tr2 = tq[:, 256:512]
        tr_(tr2[:, 0:128], gate_tm[:, 0:128], idb_sb)
        tr_(tr2[:, 128:256], gate_tm[:, 128:256], idb_sb)
        return tr2

    def OG(c, tr2, big):
        # gateT eviction on ACT: it has ~2us of slack before og needs it,
        # and the DVE is the busier queue in steady state
        gateT = work.tile([128, DV], F32, tag="gT")
        nc.scalar.copy(gateT[:], tr2[:])
        og = work.tile([128, DV], BF16, tag="og")
        nc.vector.tensor_mul(og[:], big[:, 0:256], gateT[:])
        return og

    def FINmm(c, og):
        fin = ppool.tile([128, 1024], F32, tag="proj")
        for nb in range(2):
            cols = slice(nb * 512, (nb + 1) * 512)
            mm(fin[:, cols], lhsT=og[:, 0:128],
               rhs=wout_sb[:, 0, cols], start=True, stop=False)
            mm(fin[:, cols], lhsT=og[:, 128:256],
               rhs=wout_sb[:, 1, cols], start=False, stop=True)
        return fin

    def FINout(c, fin):
        # emitted late so w_new precedes the casts in the vector queue
        tok0 = c * C
        o_sb = outp.tile([128, 1024], BF16, tag="o")
        if c == NCH - 1:
            # parallel casts; each column half stores as soon as its own
            # cast lands (don't gate the first store on both casts)
            nc.vector.tensor_copy(o_sb[:, 0:512], fin[:, 0:512])
            nc.sync.dma_start(out=out[tok0:tok0 + C, 0:512],
                              in_=o_sb[:, 0:512])
            nc.scalar.copy(o_sb[:, 512:1024], fin[:, 512:1024])
            nc.scalar.dma_start(out=out[tok0:tok0 + C, 512:1024],
                                in_=o_sb[:, 512:1024])
        else:
            nc.vector.tensor_copy(o_sb[:, 0:512], fin[:, 0:512])
            nc.vector.tensor_copy(o_sb[:, 512:1024], fin[:, 512:1024])
            nc.sync.dma_start(out=out[tok0:tok0 + C, :], in_=o_sb[:])

    # ---- pipeline driver ----
    state = {"w_prev": None, "w_prev_for_o": None}
    pend = {}
    prv = None   # chunk c-1's (ug, rf, big, r) for the interleaved output tail

    # Prologue: project chunks 0 AND 1's bank1 while wb0/x stream in — the
    # PE would otherwise idle ~3us waiting for the q|k|v weight quarters.
    # The loop then emits P1(c+1) only from iteration 1 on, preserving the
    # proj/fin ppool rotation (depth stays 2 - no forward-wait deadlock).
    proj0, e1_0 = P1(0)
    u1_0 = SPb(0, e1_0)
    g_0 = SPc(0, u1_0)
    proj1, e1_1 = P1(1)
    u1_1 = SPb(1, e1_1)
    g_1 = SPc(1, u1_1)
    P0(0, proj0)
    qk0, v0, ug0 = QK(0, proj0)
    pend[0] = dict(proj=proj0, g=g_0, qk=qk0, v=v0, ug=ug0)
    pend[1] = dict(proj=proj1, g=g_1)

    for c in range(NCH):
        p = pend[c]
        proj, g_c = p["proj"], p["g"]
        if prv is not None:
            r_p = R(c - 1, prv["ssq"])
        sml, bmm = Bmm(c, proj, g_c)
        En_tok, ET, EnT, f_vec, eg = Ex(c, proj, sml, bmm)
        # qk/v/ug were evicted LAST iteration (right after P0(c) closed):
        # T(c) below never waits on the DVE for the qk cast
        qk_sb, v_tm, ug = p["qk"], p["v"], p["ug"]
        if prv is not None:
            gate_p = GATE(c - 1, prv["ug"], prv["rf"], r_p)
        # transposes run before the c+1 projections on the in-order PE so
        # the DVE products (qtT/ktT/kt_tm/og) are all ready long before A
        tq = T(c, qk_sb)
        if prv is not None:
            tr2_p = TR2(c - 1, gate_p, tq)
        qtT, ktT, kt_tm = M(c, tq, qk_sb, En_tok, ET, EnT)
        if prv is not None:
            og_p = OG(c - 1, tr2_p, prv["big"])
        if c + 1 < NCH:
            if (c + 1) not in pend:
                projn, e1n = P1(c + 1)
                u1n = SPb(c + 1, e1n)
                P0(c + 1, projn)
                gn = SPc(c + 1, u1n)
                pend[c + 1] = dict(proj=projn, g=gn)
            else:
                projn = pend[c + 1]["proj"]
                P0(c + 1, projn)
            qkn, vn, ugn = QK(c + 1, projn)
            pend[c + 1].update(qk=qkn, v=vn, ug=ugn)
        A(c, sml, qtT, ktT)
        at_m = AM(c, sml)
        rf = SILU(c, eg)
        state["w_prev_for_o"] = state["w_prev"]
        # OT before FINmm: OT's at_m is ready right after A, while FIN's og
        # arrives later - this order gives the og chain ~0.7us more slack
        big = OT(c, at_m, qtT, v_tm)
        if prv is not None:
            fin_p = FINmm(c - 1, og_p)
        sq = SQ(c, big)
        ST(c, big, kt_tm, v_tm, f_vec)
        ssq_c = SSQ(c, sq, sml)
        if prv is not None:
            FINout(c - 1, fin_p)
        prv = dict(ug=ug, rf=rf, big=big, ssq=ssq_c)

    # last chunk's output tail
    r_p = R(NCH - 1, prv["ssq"])
    gate_p = GATE(NCH - 1, prv["ug"], prv["rf"], r_p)
    t_last = pbf.tile([128, 512], BF16, tag="tqk")
    tr2_p = TR2(NCH - 1, gate_p, t_last)
    og_p = OG(NCH - 1, tr2_p, prv["big"])
    fin_l = FINmm(NCH - 1, og_p)
    FINout(NCH - 1, fin_l)

def _build_nc():
    _patch_act_tables()
    nc = bacc.Bacc("TRN2", target_bir_lowering=False, debug=False, num_devices=8)
    ap = {
        "xT": nc.dram_tensor("xT", [128, N, NK], BF16, kind="ExternalInput").ap(),
        "wb1": nc.dram_tensor("wb1", [128, NK, 384], BF16,
                              kind="ExternalInput").ap(),
        "wb0": nc.dram_tensor("wb0", [128, NK, 512], BF16,
                              kind="ExternalInput").ap(),
        "woutT": nc.dram_tensor("woutT", [2, 128, D], BF16,
                                kind="ExternalInput").ap(),
        "bgk2": nc.dram_tensor("bgk2", [1, 128], BF16, kind="ExternalInput").ap(),
        "cb": nc.dram_tensor("cb", [128, 256], BF16, kind="ExternalInput").ap(),
        "out": nc.dram_tensor("out", [N, D], BF16, kind="ExternalOutput").ap(),
    }
    with tile.TileContext(nc) as tc:
        with ExitStack() as ctx:
            _emit_kernel(ctx, tc, ap)
    nc.compile()
    return nc


def kernel(x, Wq, Wk, Wv, Wg, Wgk1, Wgk2, bgk2, Wout, rms_w):
    global LAST_RESULTS
    BF = ml_dtypes.bfloat16
    x = np.asarray(x, np.float32)
    Wz = (np.asarray(Wgk1, np.float32) @ np.asarray(Wgk2, np.float32))
    L = np.triu(np.ones((C, C), np.float32))
    I32 = np.eye(128, dtype=np.float32)
    cb = np.ascontiguousarray(
        np.concatenate([L, I32], axis=1)).astype(BF)

    in_maps = []
    for core in range(8):
        b, h = core // H, core % H
        xTb = np.ascontiguousarray(
            x[b].T.reshape(NK, 128, N).transpose(1, 2, 0)).astype(BF)
        blob = np.concatenate([
            Wq[:, h * DK:(h + 1) * DK], Wk[:, h * DK:(h + 1) * DK],
            Wv[:, h * DV:(h + 1) * DV], Wg[:, h * DV:(h + 1) * DV],
            Wz[:, h * DK:(h + 1) * DK]],
            axis=1).astype(np.float32).reshape(NK, 128, 896)
        wb0 = np.ascontiguousarray(
            blob[:, :, 0:512].transpose(1, 0, 2)).astype(BF)
        wb1 = np.ascontiguousarray(
            blob[:, :, 512:896].transpose(1, 0, 2)).astype(BF)
        woutP = np.ascontiguousarray(
            (np.asarray(rms_w, np.float32)[:, None]
             * np.asarray(Wout, np.float32)[h * DV:(h + 1) * DV])
        ).reshape(2, 128, D).astype(BF)
        in_maps.append({
            "xT": xTb,
            "wb1": wb1,
            "wb0": wb0,
            "woutT": woutP,
            "bgk2": np.ascontiguousarray(
                np.asarray(bgk2, np.float32)[h * DK:(h + 1) * DK][None, :]
            ).astype(BF),
            "cb": cb,
        })

    nc = _build_nc()
    trace = os.environ.get("BASSGLA_TRACE", "0") == "1"
    res = run_bass_kernel_spmd(nc, in_maps, list(range(8)), trace=trace)
    LAST_RESULTS = res

    out = np.zeros((B, N, D), np.float32)
    for core in range(8):
        out[core // H] += np.asarray(res.results[core]["out"], np.float32)
    return out


# revision 55
# speedup vs baseline: 1.0147x; 1.0147x over previous
"""Gated Linear Attention on 8 Trainium2 NeuronCores.

Sharding: one (batch, head) pair per core (B=2 x H=4 = 8 cores). Each core
computes its head's full pipeline and emits a partial [N, D] output (bf16);
the host sums the 4 head partials per batch in f32.

v6 design (89.6us -> 83.3us vs v4):
  * All heavy matmuls in bf16 (1 PE cycle/row vs 4 for fp32); PSUM accums f32.
  * Per-chunk LOCAL decay (no global cumsum carry chain): within chunk c,
    b = L^T g'' (local inclusive cumsum). q~=q*exp(-b/16), k~=k*exp(+b/16);
    cross-chunk state rescaled once per chunk by the per-feature factor
    f = exp(-b_last/16):  W_c = diag(f) (W_{c-1} + k~^T v).
  * Weight blob split into bank1 (gate|z, 384 cols) and bank0 (q|k|v, 512
    cols) DRAM tensors, quartered across the three DMA rings so bank1 +
    x chunk 0 land first: P1(0) starts ~12us instead of ~21us; wb0
    quarters pace P0(0) behind it. Input load is bandwidth-floored at
    ~300GB/s aggregate - only the ORDER is tunable.
  * Prologue projects chunks 0 AND 1's bank1 during the load window; QK
    evictions are hoisted one iteration early (right after P0(c+1)
    closes) so T(c) never waits on the DVE.
  * Per-chunk PE order: Bmm, T/TR2 transposes FIRST, then P1/P0(c+1),
    A, OT before FINmm(c-1) (OT's at_m is ready early; the swap shifts
    each chunk's SQ/SSQ/R tail ~1us earlier, relaxing the cross-chunk
    R->GATE->TR2->og chain that feeds FIN).
  * PSUM: sml (bT|at|ssq) double-buffered - load-bearing; tq|tr2 packed
    into ONE bank ([128,512] tile, 4 transposes per iteration).
  * Consts packed into one small DMA (Lb|idb); L (f32) derived on-chip.
  * Warmup = 8 matmuls (ends right as the first wb1 quarter lands).
  * All mid-loop stores on the sync ring (a store issue on the scalar
    ring delays the ACT queue); last chunk's store column-split with
    per-half cast->issue on vector/scalar + sync/scalar rings.
  * ACT table discipline: exp+ln resolve to the combined table by blanking
    the exp-only/ln-only sets for the load-insertion pass (ids still index
    the real act_info.json). Silu via reciprocal_approx_fast. 2 loads total.
  * RMS r deferred and folded into the silu gate; gateT eviction on ACT;
    bf16 I/O; contiguous 2KB+ DMA rows. GpSimd/Pool is used ONLY for DMA
    issue - any Pool compute op on the chunk path costs multi-us dispatch.
"""

import os
from contextlib import ExitStack

import numpy as np
import ml_dtypes

import concourse.bass as bass
import concourse.tile as tile
from concourse import bacc, mybir
from concourse.tile_rust import add_dep_helper
from concourse.bass_utils import run_bass_kernel_spmd

F32 = mybir.dt.float32
BF16 = mybir.dt.bfloat16
AF = mybir.ActivationFunctionType
ALU = mybir.AluOpType

B, N, D, H = 2, 1024, 1024, 4
KD, VD, DK, DV = 512, 1024, 128, 256
C = 128                    # chunk length (= token partitions)
NCH = N // C               # 8 chunks
NK = D // 128              # 8 contraction tiles
GLN = 16.0
EPS = 1e-5
E48 = float(np.exp(48.0).astype(np.float32))

# module-level stash so test.py can grab profiling results
LAST_RESULTS = None

_BLANK_TABLES = ("exp_and_others", "natural_log", "exp_and_friends")
_tables_patched = False


def _patch_act_tables():
    """Steer the ACT-table-load chooser toward natural_log_exp_and_others so
    exp+ln never alternate table loads. Only the (name -> funcs) map used by
    the load-insertion pass and CoreSim is filtered; emitted act_func_set_ids
    still index the real act_info.json, so walrus/hardware see valid sets."""
    global _tables_patched
    if _tables_patched:
        return
    _tables_patched = True
    from concourse import hw_specs, bass_interp
    orig = hw_specs.get_activation_tables

    def patched(arch):
        tabs = dict(orig(arch))
        for name in _BLANK_TABLES:
            if name in tabs:
                tabs[name] = set()
        return tabs

    bacc.get_activation_tables = patched
    bass_interp.get_activation_tables = patched



def _emit_kernel(ctx: ExitStack, tc: "tile.TileContext", ap: dict):
    nc = tc.nc

    # Chain all PE instructions in program order (PE executes in-order; this
    # keeps the Tile scheduler from reordering matmuls within a PSUM bank,
    # which would break has_written clear ordering).
    pe_prev = [None]

    def mm(*args, **kw):
        inst = nc.tensor.matmul(*args, **kw)
        if kw.get("skip_group_check") or kw.get("start") in (False, None):
            # keep explicit order only for matmuls that join open psum groups
            if pe_prev[0] is not None:
                add_dep_helper(inst.ins, pe_prev[0], sync=False,
                               reason="pe-order")
        pe_prev[0] = inst.ins
        return inst

    def tr_(out, in_, ident):
        inst = nc.tensor.transpose(out, in_, ident)
        if pe_prev[0] is not None:
            add_dep_helper(inst.ins, pe_prev[0], sync=False, reason="pe-order")
        pe_prev[0] = inst.ins
        return inst

    xT, wb1d, wb0d, woutT = ap["xT"], ap["wb1"], ap["wb0"], ap["woutT"]
    bgk2, cb = ap["bgk2"], ap["cb"]
    out = ap["out"]

    consts = ctx.enter_context(tc.tile_pool(name="consts", bufs=1))
    wpool = ctx.enter_context(tc.tile_pool(name="wpool", bufs=1))
    work = ctx.enter_context(tc.tile_pool(name="work", bufs=3))
    wst = ctx.enter_context(tc.tile_pool(name="wst", bufs=2))
    outp = ctx.enter_context(tc.tile_pool(name="outp", bufs=3))
    ppool = ctx.enter_context(tc.tile_pool(name="ppool", bufs=2, space="PSUM"))
    ptr = ctx.enter_context(tc.tile_pool(name="ptr", bufs=2, space="PSUM"))
    pbf = ctx.enter_context(tc.tile_pool(name="pbf", bufs=1, space="PSUM"))
    pao = ctx.enter_context(tc.tile_pool(name="pao", bufs=1, space="PSUM"))

    # PE clock warmup source: FIRST vector instruction so the warmup matmuls
    # can launch while the DMA rings fill. The tensor engine needs ~3us of
    # continuous execution to reach max frequency.
    warm = consts.tile([128, 512], BF16)
    nc.vector.memset(warm[:], 0.0)

    # ---- DMA schedule: need-ordered across the three ~110GB/s rings.
    # Ring layout (issue order = per-engine program order):
    #   gpsimd: x0 | cb(Lb|idb) | x1 | x2 | x3 | x6
    #   sync:   wb1[0:4] | wb0[0:2] | wb0[4:6] | wout0 | x4
    #   scalar: bgk2 | wb1[4:8] | wb0[2:4] | wb0[6:8] | wout1 | x5 | x7
    # bank1 (gate|z) + x0 land ~10.5us -> P1(0) starts; wb0 quarters pace
    # P0(0) to ~15.5us; later x chunks have chunk-pipeline slack.
    xsb = wpool.tile([128, N, NK], BF16)
    wb1_sb = wpool.tile([128, NK, 384], BF16)
    wb0_sb = wpool.tile([128, NK, 512], BF16)
    wout_sb = wpool.tile([128, 2, D], BF16)
    cb_sb = consts.tile([128, 256], BF16)
    bg_sb = consts.tile([1, 128], BF16)

    nc.gpsimd.dma_start(out=xsb[:, 0:C, :], in_=xT[:, 0:C, :])
    nc.scalar.dma_start(out=bg_sb[:], in_=bgk2[:])
    nc.sync.dma_start(out=wb1_sb[:, 0:2, :], in_=wb1d[:, 0:2, :])
    nc.scalar.dma_start(out=wb1_sb[:, 2:4, :], in_=wb1d[:, 2:4, :])
    nc.sync.dma_start(out=wb1_sb[:, 4:6, :], in_=wb1d[:, 4:6, :])
    nc.scalar.dma_start(out=wb1_sb[:, 6:8, :], in_=wb1d[:, 6:8, :])
    nc.gpsimd.dma_start(out=cb_sb[:], in_=cb[:])
    nc.sync.dma_start(out=wb0_sb[:, 0:2, :], in_=wb0d[:, 0:2, :])
    nc.scalar.dma_start(out=wb0_sb[:, 2:4, :], in_=wb0d[:, 2:4, :])
    nc.gpsimd.dma_start(out=xsb[:, C:2 * C, :], in_=xT[:, C:2 * C, :])
    nc.sync.dma_start(out=wb0_sb[:, 4:6, :], in_=wb0d[:, 4:6, :])
    nc.scalar.dma_start(out=wb0_sb[:, 6:8, :], in_=wb0d[:, 6:8, :])
    nc.gpsimd.dma_start(out=xsb[:, 2 * C:3 * C, :], in_=xT[:, 2 * C:3 * C, :])
    nc.sync.dma_start(out=wout_sb[:, 0, :], in_=woutT[0])
    nc.scalar.dma_start(out=wout_sb[:, 1, :], in_=woutT[1])
    nc.gpsimd.dma_start(out=xsb[:, 3 * C:4 * C, :], in_=xT[:, 3 * C:4 * C, :])
    nc.sync.dma_start(out=xsb[:, 4 * C:5 * C, :], in_=xT[:, 4 * C:5 * C, :])
    nc.scalar.dma_start(out=xsb[:, 5 * C:6 * C, :], in_=xT[:, 5 * C:6 * C, :])
    nc.gpsimd.dma_start(out=xsb[:, 6 * C:7 * C, :], in_=xT[:, 6 * C:7 * C, :])
    nc.scalar.dma_start(out=xsb[:, 7 * C:8 * C, :], in_=xT[:, 7 * C:8 * C, :])

    # remaining consts on vector (idle during the load phase)
    ones_row = consts.tile([1, 128], BF16)
    nc.vector.memset(ones_row[:], 1.0)
    ones_col = consts.tile([128, 1], BF16)
    nc.vector.memset(ones_col[:], 1.0)
    eps_sb = consts.tile([128, 1], F32)
    nc.vector.memset(eps_sb[:], EPS)
    Lb_sb = cb_sb[:, 0:128]               # L[s,t]=1 iff s<=t (triu), bf16
    idb_sb = cb_sb[:, 128:256]            # identity, bf16
    L_sb = consts.tile([128, 128], F32)
    nc.vector.tensor_copy(L_sb[:], cb_sb[:, 0:128])

    # PE clock warmup: dummy matmuls while the first DMAs land.
    wps = pao.tile([128, 512], F32, tag="big")
    for i in range(8):
        mm(wps[:], lhsT=warm[:, 0:128], rhs=warm[:],
           start=(i == 0), stop=(i == 7))

    # ---- main loop ---------------------------------------------------------
    # proj psum [128,1024]: bank0 {q 0:128 | k 128:256 | v 256:512}
    # bank1 {gate 512:768 | z 768:896 | b_loc 896:1024}. bank1 (and its bias
    # close) is emitted BEFORE bank0 so softplus overlaps the qkv matmuls.
    # b (token-major) and bT (feature-major) are both produced directly by
    # matmuls against the triangular mask (b = L^T g, bT = g^T L).
    # The ENTIRE output path (silu gate via reciprocal_approx_fast - no
    # activation-table switch - RMS scale, final projection, store) is inlined
    # per chunk, one chunk behind the front of the pipeline, so outputs
    # stream to HBM throughout the loop and no drain phase remains.

    def P1(c):
        proj = ppool.tile([128, 1024], F32, tag="proj")
        tok = slice(c * C, (c + 1) * C)
        for k in range(NK):
            mm(proj[:, 512:896], lhsT=xsb[:, tok, k], rhs=wb1_sb[:, k, :],
               start=(k == 0), stop=False)
        bias_mm = mm(proj[:, 768:896], lhsT=ones_row[:], rhs=bg_sb[:],
                     start=False, stop=True)
        # softplus part a: e1 = exp(-z)
        e1 = work.tile([128, 128], F32, tag="e1")
        i = nc.scalar.activation(e1[:], proj[:, 768:896], AF.Exp, scale=-1.0)
        add_dep_helper(i.ins, bias_mm.ins, sync=False, reason="z after close")
        return proj, e1

    def SPb(c, e1):
        u1 = work.tile([128, 128], F32, tag="u1")
        nc.vector.tensor_scalar(u1[:], e1[:], 1.0, E48, ALU.add, ALU.min)
        return u1

    def SPc(c, u1):
        g_c = work.tile([128, 128], BF16, tag="g")
        nc.scalar.activation(g_c[:], u1[:], AF.Ln)
        return g_c

    def P0(c, proj):
        tok = slice(c * C, (c + 1) * C)
        for k in range(NK):
            mm(proj[:, 0:512], lhsT=xsb[:, tok, k], rhs=wb0_sb[:, k, :],
               start=(k == 0), stop=(k == NK - 1))

    def Bmm(c, proj, g_c):
        bmm = mm(proj[:, 896:1024], lhsT=Lb_sb, rhs=g_c[:],
                 start=False, stop=False, skip_group_check=True)
        sml = ptr.tile([128, 512], F32, tag="sml")   # bT | at | ssq
        mm(sml[:, 0:128], lhsT=g_c[:], rhs=Lb_sb, start=True, stop=True)
        return sml, bmm

    def Ex(c, proj, sml, bmm):
        En_tok = work.tile([128, 128], BF16, tag="Ent")
        i = nc.scalar.activation(En_tok[:], proj[:, 896:1024], AF.Exp,
                                 scale=1.0 / GLN)
        add_dep_helper(i.ins, bmm.ins, sync=False, reason="b after b-mm")
        ET = work.tile([128, 128], BF16, tag="ET")
        nc.scalar.activation(ET[:], sml[:, 0:128], AF.Exp, scale=-1.0 / GLN)
        EnT = work.tile([128, 128], BF16, tag="EnT")
        nc.scalar.activation(EnT[:], sml[:, 0:128], AF.Exp, scale=1.0 / GLN)
        f_vec = work.tile([128, 1], F32, tag="f")
        nc.scalar.activation(f_vec[:], sml[:, 127:128], AF.Exp, scale=-1.0 / GLN)
        # silu ingredient: eg = exp(-ug) straight from psum
        eg = work.tile([128, DV], F32, tag="eg")
        nc.scalar.activation(eg[:], proj[:, 512:768], AF.Exp, scale=-1.0)
        return En_tok, ET, EnT, f_vec, eg

    def QK(c, proj):
        qk_sb = work.tile([128, 256], BF16, tag="qk")
        nc.vector.tensor_copy(qk_sb[:], proj[:, 0:256])
        v_tm = work.tile([128, DV], BF16, tag="v")
        nc.scalar.copy(v_tm[:], proj[:, 256:512])
        ug = work.tile([128, DV], F32, tag="ug")
        nc.scalar.copy(ug[:], proj[:, 512:768])
        return qk_sb, v_tm, ug

    def T(c, qk_sb):
        # one psum bank holds this iteration's 4 transposes: qT|kT|gateT(c-1)
        tq = pbf.tile([128, 512], BF16, tag="tqk")
        tr_(tq[:, 0:128], qk_sb[:, 0:128], idb_sb)
        tr_(tq[:, 128:256], qk_sb[:, 128:256], idb_sb)
        return tq

    def M(c, tq, qk_sb, En_tok, ET, EnT):
        qtT = work.tile([128, 128], BF16, tag="qtT")
        nc.vector.tensor_mul(qtT[:], tq[:, 0:128], ET[:])
        ktT = work.tile([128, 128], BF16, tag="ktT")
        nc.vector.tensor_mul(ktT[:], tq[:, 128:256], EnT[:])
        kt_tm = work.tile([128, 128], BF16, tag="kt")
        nc.vector.tensor_mul(kt_tm[:], qk_sb[:, 128:256], En_tok[:])
        return qtT, ktT, kt_tm

    def SILU(c, eg):
        # silu: rf = 1/(1+eg); emitted after at_m so the critical DVE ops
        # (qtT/ktT/kt_tm/og/at) run first. Pool engine is NOT used for
        # per-chunk ops: its dispatch latency is multi-us.
        dg = work.tile([128, DV], F32, tag="dg")
        nc.vector.tensor_scalar_add(dg[:], eg[:], 1.0)
        rf = work.tile([128, DV], F32, tag="rf")
        nc.vector.reciprocal_approx_fast(rf[:], dg[:])
        return rf

    def A(c, sml, qtT, ktT):
        mm(sml[:, 128:256], lhsT=ktT[:], rhs=qtT[:], start=True, stop=True)

    def AM(c, sml):
        at_m = work.tile([128, 128], BF16, tag="atm")
        nc.vector.tensor_mul(at_m[:], sml[:, 128:256], L_sb[:])
        return at_m

    def OT(c, at_m, qtT, v_tm):
        big = pao.tile([128, 512], F32, tag="big")
        ot = big[:, 0:256]
        if c > 0:
            w_prev = state["w_prev_for_o"]
            mm(ot[:, 0:128], lhsT=w_prev[:, 0:128], rhs=qtT[:],
               start=True, stop=False)
            mm(ot[:, 128:256], lhsT=w_prev[:, 128:256], rhs=qtT[:],
               start=False, stop=False, skip_group_check=True)
            mm(ot[:, 0:128], lhsT=v_tm[:, 0:128], rhs=at_m[:],
               start=False, stop=False, skip_group_check=True)
        else:
            mm(ot[:, 0:128], lhsT=v_tm[:, 0:128], rhs=at_m[:],
               start=True, stop=False)
        mm(ot[:, 128:256], lhsT=v_tm[:, 128:256], rhs=at_m[:],
           start=False, stop=False, skip_group_check=True)
        return big

    def ST(c, big, kt_tm, v_tm, f_vec):
        if c == NCH - 1:
            return   # final state is never consumed
        st = big[:, 256:512]
        mm(st[:], lhsT=kt_tm[:], rhs=v_tm[:], start=True, stop=False,
           skip_group_check=True)
        if c > 0:
            mm(st[:], lhsT=idb_sb, rhs=state["w_prev"][:], start=False,
               stop=False, skip_group_check=True)
        w_new = wst.tile([128, DV], BF16, tag="w")
        nc.vector.tensor_scalar(w_new[:], st[:], f_vec[:], None, ALU.mult)
        state["w_prev"] = w_new

    def SQ(c, big):
        sq = work.tile([128, DV], BF16, tag="sq")
        nc.scalar.square(sq[:], big[:, 0:256])
        return sq

    def SSQ(c, sq, sml):
        ssq = sml[:, 256:257]
        mm(ssq, lhsT=sq[:, 0:128], rhs=ones_col[:],
           start=True, stop=False, skip_group_check=True)
        mm(ssq, lhsT=sq[:, 128:256], rhs=ones_col[:],
           start=False, stop=False, skip_group_check=True)
        return ssq

    def R(c, ssq):
        s_c = work.tile([128, 1], F32, tag="s")
        nc.scalar.activation(s_c[:], ssq, AF.Ln, scale=1.0 / DV, bias=eps_sb[:])
        r_c = work.tile([128, 1], F32, tag="r")
        nc.scalar.activation(r_c[:], s_c[:], AF.Exp, scale=-0.5)
        return r_c

    def GATE(c, ug, rf, r_c):
        # gate*r = (ug*r) * sigmoid(ug), sigmoid via fast reciprocal
        gate_tm = work.tile([128, DV], BF16, tag="gate")
        nc.vector.scalar_tensor_tensor(gate_tm[:], ug[:], r_c[:], rf[:],
                                       ALU.mult, ALU.mult)
        return gate_tm

    def TR2(c, gate_tm, tq):
        tr2 = tq[:, 256:512]
        tr_(tr2[:, 0:128], gate_tm[:, 0:128], idb_sb)
        tr_(tr2[:, 128:256], gate_tm[:, 128:256], idb_sb)
        return tr2

    def OG(c, tr2, big):
        # gateT eviction on ACT: it has ~2us of slack before og needs it,
        # and the DVE is the busier queue in steady state
        gateT = work.tile([128, DV], F32, tag="gT")
        nc.scalar.copy(gateT[:], tr2[:])
        og = work.tile([128, DV], BF16, tag="og")
        nc.vector.tensor_mul(og[:], big[:, 0:256], gateT[:])
        return og

    def FINmm(c, og):
        fin = ppool.tile([128, 1024], F32, tag="proj")
        for nb in range(2):
            cols = slice(nb * 512, (nb + 1) * 512)
            mm(fin[:, cols], lhsT=og[:, 0:128],
               rhs=wout_sb[:, 0, cols], start=True, stop=False)
            mm(fin[:, cols], lhsT=og[:, 128:256],
               rhs=wout_sb[:, 1, cols], start=False, stop=True)
        return fin

    def FINout(c, fin):
        # emitted late so w_new precedes the casts in the vector queue
        tok0 = c * C
        o_sb = outp.tile([128, 1024], BF16, tag="o")
        if c == NCH - 1:
            # parallel casts; each column half stores as soon as its own
            # cast lands (don't gate the first store on both casts)
            nc.vector.tensor_copy(o_sb[:, 0:512], fin[:, 0:512])
            nc.sync.dma_start(out=out[tok0:tok0 + C, 0:512],
                              in_=o_sb[:, 0:512])
            nc.scalar.copy(o_sb[:, 512:1024], fin[:, 512:1024])
            nc.scalar.dma_start(out=out[tok0:tok0 + C, 512:1024],
                                in_=o_sb[:, 512:1024])
        else:
            nc.vector.tensor_copy(o_sb[:, 0:512], fin[:, 0:512])
            nc.vector.tensor_copy(o_sb[:, 512:1024], fin[:, 512:1024])
            nc.sync.dma_start(out=out[tok0:tok0 + C, :], in_=o_sb[:])

    # ---- pipeline driver ----
    state = {"w_prev": None, "w_prev_for_o": None}
    pend = {}
    prv = None   # chunk c-1's (ug, rf, big, r) for the interleaved output tail

    # Prologue: project chunks 0 AND 1's bank1 while wb0/x stream in — the
    # PE would otherwise idle ~3us waiting for the q|k|v weight quarters.
    # The loop then emits P1(c+1) only from iteration 1 on, preserving the
    # proj/fin ppool rotation (depth stays 2 - no forward-wait deadlock).
    proj0, e1_0 = P1(0)
    u1_0 = SPb(0, e1_0)
    g_0 = SPc(0, u1_0)
    proj1, e1_1 = P1(1)
    u1_1 = SPb(1, e1_1)
    g_1 = SPc(1, u1_1)
    P0(0, proj0)
    qk0, v0, ug0 = QK(0, proj0)
    pend[0] = dict(proj=proj0, g=g_0, qk=qk0, v=v0, ug=ug0)
    pend[1] = dict(proj=proj1, g=g_1)

    for c in range(NCH):
        p = pend[c]
        proj, g_c = p["proj"], p["g"]
        if prv is not None:
            r_p = R(c - 1, prv["ssq"])
        sml, bmm = Bmm(c, proj, g_c)
        En_tok, ET, EnT, f_vec, eg = Ex(c, proj, sml, bmm)
        # qk/v/ug were evicted LAST iteration (right after P0(c) closed):
        # T(c) below never waits on the DVE for the qk cast
        qk_sb, v_tm, ug = p["qk"], p["v"], p["ug"]
        if prv is not None:
            gate_p = GATE(c - 1, prv["ug"], prv["rf"], r_p)
        # transposes run before the c+1 projections on the in-order PE so
        # the DVE products (qtT/ktT/kt_tm/og) are all ready long before A
        tq = T(c, qk_sb)
        if prv is not None:
            tr2_p = TR2(c - 1, gate_p, tq)
        qtT, ktT, kt_tm = M(c, tq, qk_sb, En_tok, ET, EnT)
        if prv is not None:
            og_p = OG(c - 1, tr2_p, prv["big"])
        if c + 1 < NCH:
            if (c + 1) not in pend:
                projn, e1n = P1(c + 1)
                u1n = SPb(c + 1, e1n)
                P0(c + 1, projn)
                gn = SPc(c + 1, u1n)
                pend[c + 1] = dict(proj=projn, g=gn)
            else:
                projn = pend[c + 1]["proj"]
                P0(c + 1, projn)
            qkn, vn, ugn = QK(c + 1, projn)
            pend[c + 1].update(qk=qkn, v=vn, ug=ugn)
        A(c, sml, qtT, ktT)
        at_m = AM(c, sml)
        rf = SILU(c, eg)
        state["w_prev_for_o"] = state["w_prev"]
        # OT before FINmm: OT's at_m is ready right after A, while FIN's og
        # arrives later - this order gives the og chain ~0.7us more slack
        big = OT(c, at_m, qtT, v_tm)
        if prv is not None:
            fin_p = FINmm(c - 1, og_p)
        sq = SQ(c, big)
        ST(c, big, kt_tm, v_tm, f_vec)
        ssq_c = SSQ(c, sq, sml)
        if prv is not None:
            FINout(c - 1, fin_p)
        prv = dict(ug=ug, rf=rf, big=big, ssq=ssq_c)

    # last chunk's output tail
    r_p = R(NCH - 1, prv["ssq"])
    gate_p = GATE(NCH - 1, prv["ug"], prv["rf"], r_p)
    t_last = pbf.tile([128, 512], BF16, tag="tqk")
    tr2_p = TR2(NCH - 1, gate_p, t_last)
    og_p = OG(NCH - 1, tr2_p, prv["big"])
    fin_l = FINmm(NCH - 1, og_p)
    FINout(NCH - 1, fin_l)

def _build_nc():
    _patch_act_tables()
    nc = bacc.Bacc("TRN2", target_bir_lowering=False, debug=False, num_devices=8)
    ap = {
        "xT": nc.dram_tensor("xT", [128, N, NK], BF16, kind="ExternalInput").ap(),
        "wb1": nc.dram_tensor("wb1", [128, NK, 384], BF16,
                              kind="ExternalInput").ap(),
        "wb0": nc.dram_tensor("wb0", [128, NK, 512], BF16,
                              kind="ExternalInput").ap(),
        "woutT": nc.dram_tensor("woutT", [2, 128, D], BF16,
                                kind="ExternalInput").ap(),
        "bgk2": nc.dram_tensor("bgk2", [1, 128], BF16, kind="ExternalInput").ap(),
        "cb": nc.dram_tensor("cb", [128, 256], BF16, kind="ExternalInput").ap(),
        "out": nc.dram_tensor("out", [N, D], BF16, kind="ExternalOutput").ap(),
    }
    with tile.TileContext(nc) as tc:
        with ExitStack() as ctx:
            _emit_kernel(ctx, tc, ap)
    nc.compile()
    return nc


def kernel(x, Wq, Wk, Wv, Wg, Wgk1, Wgk2, bgk2, Wout, rms_w):
    global LAST_RESULTS
    BF = ml_dtypes.bfloat16
    x = np.asarray(x, np.float32)
    Wz = (np.asarray(Wgk1, np.float32) @ np.asarray(Wgk2, np.float32))
    L = np.triu(np.ones((C, C), np.float32))
    I32 = np.eye(128, dtype=np.float32)
    cb = np.ascontiguousarray(
        np.concatenate([L, I32], axis=1)).astype(BF)

    in_maps = []
    for core in range(8):
        b, h = core // H, core % H
        xTb = np.ascontiguousarray(
            x[b].T.reshape(NK, 128, N).transpose(1, 2, 0)).astype(BF)
        blob = np.concatenate([
            Wq[:, h * DK:(h + 1) * DK], Wk[:, h * DK:(h + 1) * DK],
            Wv[:, h * DV:(h + 1) * DV], Wg[:, h * DV:(h + 1) * DV],
            Wz[:, h * DK:(h + 1) * DK]],
            axis=1).astype(np.float32).reshape(NK, 128, 896)
        wb0 = np.ascontiguousarray(
            blob[:, :, 0:512].transpose(1, 0, 2)).astype(BF)
        wb1 = np.ascontiguousarray(
            blob[:, :, 512:896].transpose(1, 0, 2)).astype(BF)
        woutP = np.ascontiguousarray(
            (np.asarray(rms_w, np.float32)[:, None]
             * np.asarray(Wout, np.float32)[h * DV:(h + 1) * DV])
        ).reshape(2, 128, D).astype(BF)
        in_maps.append({
            "xT": xTb,
            "wb1": wb1,
            "wb0": wb0,
            "woutT": woutP,
            "bgk2": np.ascontiguousarray(
                np.asarray(bgk2, np.float32)[h * DK:(h + 1) * DK][None, :]
            ).astype(BF),
            "cb": cb,
        })

    nc = _build_nc()
    trace = os.environ.get("BASSGLA_TRACE", "0") == "1"
    res = run_bass_kernel_spmd(nc, in_maps, list(range(8)), trace=trace)
    LAST_RESULTS = res

    out = np.zeros((B, N, D), np.float32)
    for core in range(8):
        out[core // H] += np.asarray(res.results[core]["out"], np.float32)
    return out


# revision 56
# speedup vs baseline: 1.0234x; 1.0086x over previous
"""Gated Linear Attention on 8 Trainium2 NeuronCores.

Sharding: one (batch, head) pair per core (B=2 x H=4 = 8 cores). Each core
computes its head's full pipeline and emits a partial [N, D] output (bf16);
the host sums the 4 head partials per batch in f32.

v6 design (89.6us -> 83.3us vs v4):
  * All heavy matmuls in bf16 (1 PE cycle/row vs 4 for fp32); PSUM accums f32.
  * Per-chunk LOCAL decay (no global cumsum carry chain): within chunk c,
    b = L^T g'' (local inclusive cumsum). q~=q*exp(-b/16), k~=k*exp(+b/16);
    cross-chunk state rescaled once per chunk by the per-feature factor
    f = exp(-b_last/16):  W_c = diag(f) (W_{c-1} + k~^T v).
  * Weight blob split into bank1 (gate|z, 384 cols) and bank0 (q|k|v, 512
    cols) DRAM tensors, quartered across the three DMA rings so bank1 +
    x chunk 0 land first: P1(0) starts ~12us instead of ~21us; wb0
    quarters pace P0(0) behind it. Input load is bandwidth-floored at
    ~300GB/s aggregate - only the ORDER is tunable.
  * Prologue projects chunks 0 AND 1's bank1 during the load window; QK
    evictions are hoisted one iteration early (right after P0(c+1)
    closes) so T(c) never waits on the DVE.
  * Per-chunk PE order: Bmm, T/TR2 transposes FIRST, then P1/P0(c+1),
    A, OT before FINmm(c-1) (OT's at_m is ready early; the swap shifts
    each chunk's SQ/SSQ/R tail ~1us earlier, relaxing the cross-chunk
    R->GATE->TR2->og chain that feeds FIN).
  * PSUM: sml (bT|at|ssq) double-buffered - load-bearing; tq|tr2 packed
    into ONE bank ([128,512] tile, 4 transposes per iteration).
  * Consts packed into one small DMA (Lb|idb); L (f32) derived on-chip.
  * Warmup = 8 matmuls (ends right as the first wb1 quarter lands).
  * All mid-loop stores on the sync ring (a store issue on the scalar
    ring delays the ACT queue); last chunk's store column-split with
    per-half cast->issue on vector/scalar + sync/scalar rings.
  * ACT table discipline: exp+ln resolve to the combined table by blanking
    the exp-only/ln-only sets for the load-insertion pass (ids still index
    the real act_info.json). Silu via reciprocal_approx_fast. 2 loads total.
  * RMS r deferred and folded into the silu gate; gateT eviction on ACT;
    bf16 I/O; contiguous 2KB+ DMA rows. GpSimd/Pool is used ONLY for DMA
    issue - any Pool compute op on the chunk path costs multi-us dispatch.
"""

import os
from contextlib import ExitStack

import numpy as np
import ml_dtypes

import concourse.bass as bass
import concourse.tile as tile
from concourse import bacc, mybir
from concourse.tile_rust import add_dep_helper
from concourse.bass_utils import run_bass_kernel_spmd

F32 = mybir.dt.float32
BF16 = mybir.dt.bfloat16
AF = mybir.ActivationFunctionType
ALU = mybir.AluOpType

B, N, D, H = 2, 1024, 1024, 4
KD, VD, DK, DV = 512, 1024, 128, 256
C = 128                    # chunk length (= token partitions)
NCH = N // C               # 8 chunks
NK = D // 128              # 8 contraction tiles
GLN = 16.0
EPS = 1e-5
E48 = float(np.exp(48.0).astype(np.float32))

# module-level stash so test.py can grab profiling results
LAST_RESULTS = None

_BLANK_TABLES = ("exp_and_others", "natural_log", "exp_and_friends")
_tables_patched = False


def _patch_act_tables():
    """Steer the ACT-table-load chooser toward natural_log_exp_and_others so
    exp+ln never alternate table loads. Only the (name -> funcs) map used by
    the load-insertion pass and CoreSim is filtered; emitted act_func_set_ids
    still index the real act_info.json, so walrus/hardware see valid sets."""
    global _tables_patched
    if _tables_patched:
        return
    _tables_patched = True
    from concourse import hw_specs, bass_interp
    orig = hw_specs.get_activation_tables

    def patched(arch):
        tabs = dict(orig(arch))
        for name in _BLANK_TABLES:
            if name in tabs:
                tabs[name] = set()
        return tabs

    bacc.get_activation_tables = patched
    bass_interp.get_activation_tables = patched



def _emit_kernel(ctx: ExitStack, tc: "tile.TileContext", ap: dict):
    nc = tc.nc

    # Chain all PE instructions in program order (PE executes in-order; this
    # keeps the Tile scheduler from reordering matmuls within a PSUM bank,
    # which would break has_written clear ordering).
    pe_prev = [None]

    def mm(*args, **kw):
        inst = nc.tensor.matmul(*args, **kw)
        if kw.get("skip_group_check") or kw.get("start") in (False, None):
            # keep explicit order only for matmuls that join open psum groups
            if pe_prev[0] is not None:
                add_dep_helper(inst.ins, pe_prev[0], sync=False,
                               reason="pe-order")
        pe_prev[0] = inst.ins
        return inst

    def tr_(out, in_, ident):
        inst = nc.tensor.transpose(out, in_, ident)
        if pe_prev[0] is not None:
            add_dep_helper(inst.ins, pe_prev[0], sync=False, reason="pe-order")
        pe_prev[0] = inst.ins
        return inst

    xT, wb1d, wb0d, woutT = ap["xT"], ap["wb1"], ap["wb0"], ap["woutT"]
    bgk2, cb = ap["bgk2"], ap["cb"]
    out = ap["out"]

    consts = ctx.enter_context(tc.tile_pool(name="consts", bufs=1))
    wpool = ctx.enter_context(tc.tile_pool(name="wpool", bufs=1))
    work = ctx.enter_context(tc.tile_pool(name="work", bufs=3))
    wst = ctx.enter_context(tc.tile_pool(name="wst", bufs=2))
    outp = ctx.enter_context(tc.tile_pool(name="outp", bufs=3))
    ppool = ctx.enter_context(tc.tile_pool(name="ppool", bufs=2, space="PSUM"))
    ptr = ctx.enter_context(tc.tile_pool(name="ptr", bufs=2, space="PSUM"))
    pbf = ctx.enter_context(tc.tile_pool(name="pbf", bufs=1, space="PSUM"))
    pao = ctx.enter_context(tc.tile_pool(name="pao", bufs=1, space="PSUM"))

    # PE clock warmup source: FIRST vector instruction so the warmup matmuls
    # can launch while the DMA rings fill. The tensor engine needs ~3us of
    # continuous execution to reach max frequency.
    warm = consts.tile([128, 512], BF16)
    nc.vector.memset(warm[:], 0.0)

    # ---- DMA schedule: need-ordered across the three ~110GB/s rings.
    # Ring layout (issue order = per-engine program order):
    #   gpsimd: x0 | cb(Lb|idb) | x1 | x2 | x3 | x6
    #   sync:   wb1[0:4] | wb0[0:2] | wb0[4:6] | wout0 | x4
    #   scalar: bgk2 | wb1[4:8] | wb0[2:4] | wb0[6:8] | wout1 | x5 | x7
    # bank1 (gate|z) + x0 land ~10.5us -> P1(0) starts; wb0 quarters pace
    # P0(0) to ~15.5us; later x chunks have chunk-pipeline slack.
    xsb = wpool.tile([128, N, NK], BF16)
    wb1_sb = wpool.tile([128, NK, 384], BF16)
    wb0_sb = wpool.tile([128, NK, 512], BF16)
    wout_sb = wpool.tile([128, 2, D], BF16)
    cb_sb = consts.tile([128, 256], BF16)
    bg_sb = consts.tile([1, 128], BF16)

    nc.gpsimd.dma_start(out=xsb[:, 0:C, :], in_=xT[:, 0:C, :])
    nc.scalar.dma_start(out=bg_sb[:], in_=bgk2[:])
    nc.sync.dma_start(out=wb1_sb[:, 0:2, :], in_=wb1d[:, 0:2, :])
    nc.scalar.dma_start(out=wb1_sb[:, 2:4, :], in_=wb1d[:, 2:4, :])
    nc.sync.dma_start(out=wb1_sb[:, 4:6, :], in_=wb1d[:, 4:6, :])
    nc.scalar.dma_start(out=wb1_sb[:, 6:8, :], in_=wb1d[:, 6:8, :])
    nc.gpsimd.dma_start(out=cb_sb[:], in_=cb[:])
    nc.sync.dma_start(out=wb0_sb[:, 0:2, :], in_=wb0d[:, 0:2, :])
    nc.scalar.dma_start(out=wb0_sb[:, 2:4, :], in_=wb0d[:, 2:4, :])
    nc.gpsimd.dma_start(out=xsb[:, C:2 * C, :], in_=xT[:, C:2 * C, :])
    nc.sync.dma_start(out=wb0_sb[:, 4:6, :], in_=wb0d[:, 4:6, :])
    nc.scalar.dma_start(out=wb0_sb[:, 6:8, :], in_=wb0d[:, 6:8, :])
    nc.gpsimd.dma_start(out=xsb[:, 2 * C:3 * C, :], in_=xT[:, 2 * C:3 * C, :])
    nc.sync.dma_start(out=wout_sb[:, 0, :], in_=woutT[0])
    nc.scalar.dma_start(out=wout_sb[:, 1, :], in_=woutT[1])
    nc.gpsimd.dma_start(out=xsb[:, 3 * C:4 * C, :], in_=xT[:, 3 * C:4 * C, :])
    nc.sync.dma_start(out=xsb[:, 4 * C:5 * C, :], in_=xT[:, 4 * C:5 * C, :])
    nc.scalar.dma_start(out=xsb[:, 5 * C:6 * C, :], in_=xT[:, 5 * C:6 * C, :])
    nc.gpsimd.dma_start(out=xsb[:, 6 * C:7 * C, :], in_=xT[:, 6 * C:7 * C, :])
    nc.scalar.dma_start(out=xsb[:, 7 * C:8 * C, :], in_=xT[:, 7 * C:8 * C, :])

    # remaining consts on vector (idle during the load phase)
    ones_row = consts.tile([1, 128], BF16)
    nc.vector.memset(ones_row[:], 1.0)
    ones_col = consts.tile([128, 1], BF16)
    nc.vector.memset(ones_col[:], 1.0)
    eps_sb = consts.tile([128, 1], F32)
    nc.vector.memset(eps_sb[:], EPS)
    Lb_sb = cb_sb[:, 0:128]               # L[s,t]=1 iff s<=t (triu), bf16
    idb_sb = cb_sb[:, 128:256]            # identity, bf16
    L_sb = consts.tile([128, 128], F32)
    nc.vector.tensor_copy(L_sb[:], cb_sb[:, 0:128])

    # PE clock warmup: dummy matmuls while the first DMAs land.
    wps = pao.tile([128, 512], F32, tag="big")
    for i in range(8):
        mm(wps[:], lhsT=warm[:, 0:128], rhs=warm[:],
           start=(i == 0), stop=(i == 7))

    # ---- main loop ---------------------------------------------------------
    # proj psum [128,1024]: bank0 {q 0:128 | k 128:256 | v 256:512}
    # bank1 {gate 512:768 | z 768:896 | b_loc 896:1024}. bank1 (and its bias
    # close) is emitted BEFORE bank0 so softplus overlaps the qkv matmuls.
    # b (token-major) and bT (feature-major) are both produced directly by
    # matmuls against the triangular mask (b = L^T g, bT = g^T L).
    # The ENTIRE output path (silu gate via reciprocal_approx_fast - no
    # activation-table switch - RMS scale, final projection, store) is inlined
    # per chunk, one chunk behind the front of the pipeline, so outputs
    # stream to HBM throughout the loop and no drain phase remains.

    def P1(c):
        proj = ppool.tile([128, 1024], F32, tag="proj")
        tok = slice(c * C, (c + 1) * C)
        for k in range(NK):
            mm(proj[:, 512:896], lhsT=xsb[:, tok, k], rhs=wb1_sb[:, k, :],
               start=(k == 0), stop=False)
        bias_mm = mm(proj[:, 768:896], lhsT=ones_row[:], rhs=bg_sb[:],
                     start=False, stop=True)
        # softplus part a: e1 = exp(-z)
        e1 = work.tile([128, 128], F32, tag="e1")
        i = nc.scalar.activation(e1[:], proj[:, 768:896], AF.Exp, scale=-1.0)
        add_dep_helper(i.ins, bias_mm.ins, sync=False, reason="z after close")
        return proj, e1

    def SPb(c, e1):
        u1 = work.tile([128, 128], F32, tag="u1")
        nc.vector.tensor_scalar(u1[:], e1[:], 1.0, E48, ALU.add, ALU.min)
        return u1

    def SPc(c, u1):
        g_c = work.tile([128, 128], BF16, tag="g")
        nc.scalar.activation(g_c[:], u1[:], AF.Ln)
        return g_c

    def P0(c, proj):
        tok = slice(c * C, (c + 1) * C)
        for k in range(NK):
            mm(proj[:, 0:512], lhsT=xsb[:, tok, k], rhs=wb0_sb[:, k, :],
               start=(k == 0), stop=(k == NK - 1))

    def Bmm(c, proj, g_c):
        bmm = mm(proj[:, 896:1024], lhsT=Lb_sb, rhs=g_c[:],
                 start=False, stop=False, skip_group_check=True)
        sml = ptr.tile([128, 512], F32, tag="sml")   # bT | at | ssq
        mm(sml[:, 0:128], lhsT=g_c[:], rhs=Lb_sb, start=True, stop=True)
        return sml, bmm

    def Ex(c, proj, sml, bmm):
        En_tok = work.tile([128, 128], BF16, tag="Ent")
        i = nc.scalar.activation(En_tok[:], proj[:, 896:1024], AF.Exp,
                                 scale=1.0 / GLN)
        add_dep_helper(i.ins, bmm.ins, sync=False, reason="b after b-mm")
        ET = work.tile([128, 128], BF16, tag="ET")
        nc.scalar.activation(ET[:], sml[:, 0:128], AF.Exp, scale=-1.0 / GLN)
        EnT = work.tile([128, 128], BF16, tag="EnT")
        nc.scalar.activation(EnT[:], sml[:, 0:128], AF.Exp, scale=1.0 / GLN)
        f_vec = work.tile([128, 1], F32, tag="f")
        nc.scalar.activation(f_vec[:], sml[:, 127:128], AF.Exp, scale=-1.0 / GLN)
        # silu ingredient: eg = exp(-ug) straight from psum
        eg = work.tile([128, DV], F32, tag="eg")
        nc.scalar.activation(eg[:], proj[:, 512:768], AF.Exp, scale=-1.0)
        return En_tok, ET, EnT, f_vec, eg

    def QK(c, proj):
        qk_sb = work.tile([128, 256], BF16, tag="qk")
        nc.vector.tensor_copy(qk_sb[:], proj[:, 0:256])
        v_tm = work.tile([128, DV], BF16, tag="v")
        nc.scalar.copy(v_tm[:], proj[:, 256:512])
        ug = work.tile([128, DV], F32, tag="ug")
        nc.scalar.copy(ug[:], proj[:, 512:768])
        return qk_sb, v_tm, ug

    def T(c, qk_sb):
        # one psum bank holds this iteration's 4 transposes: qT|kT|gateT(c-1)
        tq = pbf.tile([128, 512], BF16, tag="tqk")
        tr_(tq[:, 0:128], qk_sb[:, 0:128], idb_sb)
        tr_(tq[:, 128:256], qk_sb[:, 128:256], idb_sb)
        return tq

    def M(c, tq, qk_sb, En_tok, ET, EnT):
        qtT = work.tile([128, 128], BF16, tag="qtT")
        nc.vector.tensor_mul(qtT[:], tq[:, 0:128], ET[:])
        ktT = work.tile([128, 128], BF16, tag="ktT")
        nc.vector.tensor_mul(ktT[:], tq[:, 128:256], EnT[:])
        kt_tm = work.tile([128, 128], BF16, tag="kt")
        nc.vector.tensor_mul(kt_tm[:], qk_sb[:, 128:256], En_tok[:])
        return qtT, ktT, kt_tm

    def SILU(c, eg):
        # silu: rf = 1/(1+eg); emitted after at_m so the critical DVE ops
        # (qtT/ktT/kt_tm/og/at) run first. Pool engine is NOT used for
        # per-chunk ops: its dispatch latency is multi-us.
        dg = work.tile([128, DV], F32, tag="dg")
        nc.vector.tensor_scalar_add(dg[:], eg[:], 1.0)
        rf = work.tile([128, DV], F32, tag="rf")
        nc.vector.reciprocal_approx_fast(rf[:], dg[:])
        return rf

    def A(c, sml, qtT, ktT):
        mm(sml[:, 128:256], lhsT=ktT[:], rhs=qtT[:], start=True, stop=True)

    def AM(c, sml):
        at_m = work.tile([128, 128], BF16, tag="atm")
        nc.vector.tensor_mul(at_m[:], sml[:, 128:256], L_sb[:])
        return at_m

    def OT(c, at_m, qtT, v_tm):
        big = pao.tile([128, 512], F32, tag="big")
        ot = big[:, 0:256]
        if c > 0:
            w_prev = state["w_prev_for_o"]
            mm(ot[:, 0:128], lhsT=w_prev[:, 0:128], rhs=qtT[:],
               start=True, stop=False)
            mm(ot[:, 128:256], lhsT=w_prev[:, 128:256], rhs=qtT[:],
               start=False, stop=False, skip_group_check=True)
            mm(ot[:, 0:128], lhsT=v_tm[:, 0:128], rhs=at_m[:],
               start=False, stop=False, skip_group_check=True)
        else:
            mm(ot[:, 0:128], lhsT=v_tm[:, 0:128], rhs=at_m[:],
               start=True, stop=False)
        mm(ot[:, 128:256], lhsT=v_tm[:, 128:256], rhs=at_m[:],
           start=False, stop=False, skip_group_check=True)
        return big

    def ST(c, big, kt_tm, v_tm, f_vec):
        if c == NCH - 1:
            return   # final state is never consumed
        st = big[:, 256:512]
        mm(st[:], lhsT=kt_tm[:], rhs=v_tm[:], start=True, stop=False,
           skip_group_check=True)
        if c > 0:
            mm(st[:], lhsT=idb_sb, rhs=state["w_prev"][:], start=False,
               stop=False, skip_group_check=True)
        w_new = wst.tile([128, DV], BF16, tag="w")
        nc.vector.tensor_scalar(w_new[:], st[:], f_vec[:], None, ALU.mult)
        state["w_prev"] = w_new

    def SQ(c, big):
        sq = work.tile([128, DV], BF16, tag="sq")
        nc.scalar.square(sq[:], big[:, 0:256])
        return sq

    def SSQ(c, sq, sml):
        ssq = sml[:, 256:257]
        mm(ssq, lhsT=sq[:, 0:128], rhs=ones_col[:],
           start=True, stop=False, skip_group_check=True)
        mm(ssq, lhsT=sq[:, 128:256], rhs=ones_col[:],
           start=False, stop=False, skip_group_check=True)
        return ssq

    def R(c, ssq):
        s_c = work.tile([128, 1], F32, tag="s")
        nc.scalar.activation(s_c[:], ssq, AF.Ln, scale=1.0 / DV, bias=eps_sb[:])
        r_c = work.tile([128, 1], F32, tag="r")
        nc.scalar.activation(r_c[:], s_c[:], AF.Exp, scale=-0.5)
        return r_c

    def GATE(c, ug, rf, r_c):
        # gate*r = (ug*r) * sigmoid(ug), sigmoid via fast reciprocal
        gate_tm = work.tile([128, DV], BF16, tag="gate")
        nc.vector.scalar_tensor_tensor(gate_tm[:], ug[:], r_c[:], rf[:],
                                       ALU.mult, ALU.mult)
        return gate_tm

    def TR2(c, gate_tm, tq):
        tr2 = tq[:, 256:512]
        tr_(tr2[:, 0:128], gate_tm[:, 0:128], idb_sb)
        tr_(tr2[:, 128:256], gate_tm[:, 128:256], idb_sb)
        return tr2

    def OG(c, tr2, big):
        # gateT eviction on ACT: it has ~2us of slack before og needs it,
        # and the DVE is the busier queue in steady state
        gateT = work.tile([128, DV], F32, tag="gT")
        nc.scalar.copy(gateT[:], tr2[:])
        og = work.tile([128, DV], BF16, tag="og")
        nc.vector.tensor_mul(og[:], big[:, 0:256], gateT[:])
        return og

    def FINmm(c, og):
        fin = ppool.tile([128, 1024], F32, tag="proj")
        for nb in range(2):
            cols = slice(nb * 512, (nb + 1) * 512)
            mm(fin[:, cols], lhsT=og[:, 0:128],
               rhs=wout_sb[:, 0, cols], start=True, stop=False)
            mm(fin[:, cols], lhsT=og[:, 128:256],
               rhs=wout_sb[:, 1, cols], start=False, stop=True)
        return fin

    def FINout(c, fin):
        # emitted late so w_new precedes the casts in the vector queue
        tok0 = c * C
        o_sb = outp.tile([128, 1024], BF16, tag="o")
        if c == NCH - 1:
            # parallel casts; each column half stores as soon as its own
            # cast lands (don't gate the first store on both casts)
            nc.vector.tensor_copy(o_sb[:, 0:512], fin[:, 0:512])
            nc.sync.dma_start(out=out[tok0:tok0 + C, 0:512],
                              in_=o_sb[:, 0:512])
            nc.scalar.copy(o_sb[:, 512:1024], fin[:, 512:1024])
            nc.scalar.dma_start(out=out[tok0:tok0 + C, 512:1024],
                                in_=o_sb[:, 512:1024])
        else:
            # parallel halves: two serial vector casts end ~0.7us past the
            # chunk period and stall P1(c+2) on the ppool WAR
            nc.vector.tensor_copy(o_sb[:, 0:512], fin[:, 0:512])
            nc.scalar.copy(o_sb[:, 512:1024], fin[:, 512:1024])
            nc.sync.dma_start(out=out[tok0:tok0 + C, :], in_=o_sb[:])

    # ---- pipeline driver ----
    state = {"w_prev": None, "w_prev_for_o": None}
    pend = {}
    prv = None   # chunk c-1's (ug, rf, big, r) for the interleaved output tail

    # Prologue: project chunks 0 AND 1's bank1 while wb0/x stream in — the
    # PE would otherwise idle ~3us waiting for the q|k|v weight quarters.
    # The loop then emits P1(c+1) only from iteration 1 on, preserving the
    # proj/fin ppool rotation (depth stays 2 - no forward-wait deadlock).
    proj0, e1_0 = P1(0)
    u1_0 = SPb(0, e1_0)
    g_0 = SPc(0, u1_0)
    proj1, e1_1 = P1(1)
    u1_1 = SPb(1, e1_1)
    g_1 = SPc(1, u1_1)
    P0(0, proj0)
    qk0, v0, ug0 = QK(0, proj0)
    pend[0] = dict(proj=proj0, g=g_0, qk=qk0, v=v0, ug=ug0)
    pend[1] = dict(proj=proj1, g=g_1)

    for c in range(NCH):
        p = pend[c]
        proj, g_c = p["proj"], p["g"]
        if prv is not None:
            r_p = R(c - 1, prv["ssq"])
        sml, bmm = Bmm(c, proj, g_c)
        En_tok, ET, EnT, f_vec, eg = Ex(c, proj, sml, bmm)
        # qk/v/ug were evicted LAST iteration (right after P0(c) closed):
        # T(c) below never waits on the DVE for the qk cast
        qk_sb, v_tm, ug = p["qk"], p["v"], p["ug"]
        if prv is not None:
            gate_p = GATE(c - 1, prv["ug"], prv["rf"], r_p)
        # transposes run before the c+1 projections on the in-order PE so
        # the DVE products (qtT/ktT/kt_tm/og) are all ready long before A
        tq = T(c, qk_sb)
        if prv is not None:
            tr2_p = TR2(c - 1, gate_p, tq)
        qtT, ktT, kt_tm = M(c, tq, qk_sb, En_tok, ET, EnT)
        if prv is not None:
            og_p = OG(c - 1, tr2_p, prv["big"])
        if c + 1 < NCH:
            if (c + 1) not in pend:
                projn, e1n = P1(c + 1)
                u1n = SPb(c + 1, e1n)
                P0(c + 1, projn)
                gn = SPc(c + 1, u1n)
                pend[c + 1] = dict(proj=projn, g=gn)
            else:
                projn = pend[c + 1]["proj"]
                P0(c + 1, projn)
            qkn, vn, ugn = QK(c + 1, projn)
            pend[c + 1].update(qk=qkn, v=vn, ug=ugn)
        A(c, sml, qtT, ktT)
        at_m = AM(c, sml)
        rf = SILU(c, eg)
        state["w_prev_for_o"] = state["w_prev"]
        # OT before FINmm: OT's at_m is ready right after A, while FIN's og
        # arrives later - this order gives the og chain ~0.7us more slack
        big = OT(c, at_m, qtT, v_tm)
        if prv is not None:
            fin_p = FINmm(c - 1, og_p)
        sq = SQ(c, big)
        ST(c, big, kt_tm, v_tm, f_vec)
        ssq_c = SSQ(c, sq, sml)
        if prv is not None:
            FINout(c - 1, fin_p)
        prv = dict(ug=ug, rf=rf, big=big, ssq=ssq_c)

    # last chunk's output tail
    r_p = R(NCH - 1, prv["ssq"])
    gate_p = GATE(NCH - 1, prv["ug"], prv["rf"], r_p)
    t_last = pbf.tile([128, 512], BF16, tag="tqk")
    tr2_p = TR2(NCH - 1, gate_p, t_last)
    og_p = OG(NCH - 1, tr2_p, prv["big"])
    fin_l = FINmm(NCH - 1, og_p)
    FINout(NCH - 1, fin_l)

def _build_nc():
    _patch_act_tables()
    nc = bacc.Bacc("TRN2", target_bir_lowering=False, debug=False, num_devices=8)
    ap = {
        "xT": nc.dram_tensor("xT", [128, N, NK], BF16, kind="ExternalInput").ap(),
        "wb1": nc.dram_tensor("wb1", [128, NK, 384], BF16,
                              kind="ExternalInput").ap(),
        "wb0": nc.dram_tensor("wb0", [128, NK, 512], BF16,
                              kind="ExternalInput").ap(),
        "woutT": nc.dram_tensor("woutT", [2, 128, D], BF16,
                                kind="ExternalInput").ap(),
        "bgk2": nc.dram_tensor("bgk2", [1, 128], BF16, kind="ExternalInput").ap(),
        "cb": nc.dram_tensor("cb", [128, 256], BF16, kind="ExternalInput").ap(),
        "out": nc.dram_tensor("out", [N, D], BF16, kind="ExternalOutput").ap(),
    }
    with tile.TileContext(nc) as tc:
        with ExitStack() as ctx:
            _emit_kernel(ctx, tc, ap)
    nc.compile()
    return nc


def kernel(x, Wq, Wk, Wv, Wg, Wgk1, Wgk2, bgk2, Wout, rms_w):
    global LAST_RESULTS
    BF = ml_dtypes.bfloat16
    x = np.asarray(x, np.float32)
    Wz = (np.asarray(Wgk1, np.float32) @ np.asarray(Wgk2, np.float32))
    L = np.triu(np.ones((C, C), np.float32))
    I32 = np.eye(128, dtype=np.float32)
    cb = np.ascontiguousarray(
        np.concatenate([L, I32], axis=1)).astype(BF)

    in_maps = []
    for core in range(8):
        b, h = core // H, core % H
        xTb = np.ascontiguousarray(
            x[b].T.reshape(NK, 128, N).transpose(1, 2, 0)).astype(BF)
        blob = np.concatenate([
            Wq[:, h * DK:(h + 1) * DK], Wk[:, h * DK:(h + 1) * DK],
            Wv[:, h * DV:(h + 1) * DV], Wg[:, h * DV:(h + 1) * DV],
            Wz[:, h * DK:(h + 1) * DK]],
            axis=1).astype(np.float32).reshape(NK, 128, 896)
        wb0 = np.ascontiguousarray(
            blob[:, :, 0:512].transpose(1, 0, 2)).astype(BF)
        wb1 = np.ascontiguousarray(
            blob[:, :, 512:896].transpose(1, 0, 2)).astype(BF)
        woutP = np.ascontiguousarray(
            (np.asarray(rms_w, np.float32)[:, None]
             * np.asarray(Wout, np.float32)[h * DV:(h + 1) * DV])
        ).reshape(2, 128, D).astype(BF)
        in_maps.append({
            "xT": xTb,
            "wb1": wb1,
            "wb0": wb0,
            "woutT": woutP,
            "bgk2": np.ascontiguousarray(
                np.asarray(bgk2, np.float32)[h * DK:(h + 1) * DK][None, :]
            ).astype(BF),
            "cb": cb,
        })

    nc = _build_nc()
    trace = os.environ.get("BASSGLA_TRACE", "0") == "1"
    res = run_bass_kernel_spmd(nc, in_maps, list(range(8)), trace=trace)
    LAST_RESULTS = res

    out = np.zeros((B, N, D), np.float32)
    for core in range(8):
        out[core // H] += np.asarray(res.results[core]["out"], np.float32)
    return out


# revision 57
# speedup vs baseline: 1.0274x; 1.0038x over previous
"""Gated Linear Attention on 8 Trainium2 NeuronCores.

Sharding: one (batch, head) pair per core (B=2 x H=4 = 8 cores). Each core
computes its head's full pipeline and emits a partial [N, D] output (bf16);
the host sums the 4 head partials per batch in f32.

v6 design (89.6us -> 83.3us vs v4):
  * All heavy matmuls in bf16 (1 PE cycle/row vs 4 for fp32); PSUM accums f32.
  * Per-chunk LOCAL decay (no global cumsum carry chain): within chunk c,
    b = L^T g'' (local inclusive cumsum). q~=q*exp(-b/16), k~=k*exp(+b/16);
    cross-chunk state rescaled once per chunk by the per-feature factor
    f = exp(-b_last/16):  W_c = diag(f) (W_{c-1} + k~^T v).
  * Weight blob split into bank1 (gate|z, 384 cols) and bank0 (q|k|v, 512
    cols) DRAM tensors, quartered across the three DMA rings so bank1 +
    x chunk 0 land first: P1(0) starts ~12us instead of ~21us; wb0
    quarters pace P0(0) behind it. Input load is bandwidth-floored at
    ~300GB/s aggregate - only the ORDER is tunable.
  * Prologue projects chunks 0 AND 1's bank1 during the load window; QK
    evictions are hoisted one iteration early (right after P0(c+1)
    closes) so T(c) never waits on the DVE.
  * Per-chunk PE order: Bmm, T/TR2 transposes FIRST, then P1/P0(c+1),
    A, OT before FINmm(c-1) (OT's at_m is ready early; the swap shifts
    each chunk's SQ/SSQ/R tail ~1us earlier, relaxing the cross-chunk
    R->GATE->TR2->og chain that feeds FIN).
  * PSUM: sml (bT|at|ssq) double-buffered - load-bearing; tq|tr2 packed
    into ONE bank ([128,512] tile, 4 transposes per iteration).
  * Consts packed into one small DMA (Lb|idb); L (f32) derived on-chip.
  * Warmup = 8 matmuls (ends right as the first wb1 quarter lands).
  * All mid-loop stores on the sync ring (a store issue on the scalar
    ring delays the ACT queue); last chunk's store column-split with
    per-half cast->issue on vector/scalar + sync/scalar rings.
  * ACT table discipline: exp+ln resolve to the combined table by blanking
    the exp-only/ln-only sets for the load-insertion pass (ids still index
    the real act_info.json). Silu via reciprocal_approx_fast. 2 loads total.
  * RMS r deferred and folded into the silu gate; gateT eviction on ACT;
    bf16 I/O; contiguous 2KB+ DMA rows. GpSimd/Pool is used ONLY for DMA
    issue - any Pool compute op on the chunk path costs multi-us dispatch.
"""

import os
from contextlib import ExitStack

import numpy as np
import ml_dtypes

import concourse.bass as bass
import concourse.tile as tile
from concourse import bacc, mybir
from concourse.tile_rust import add_dep_helper
from concourse.bass_utils import run_bass_kernel_spmd

F32 = mybir.dt.float32
BF16 = mybir.dt.bfloat16
AF = mybir.ActivationFunctionType
ALU = mybir.AluOpType

B, N, D, H = 2, 1024, 1024, 4
KD, VD, DK, DV = 512, 1024, 128, 256
C = 128                    # chunk length (= token partitions)
NCH = N // C               # 8 chunks
NK = D // 128              # 8 contraction tiles
GLN = 16.0
EPS = 1e-5
E48 = float(np.exp(48.0).astype(np.float32))

# module-level stash so test.py can grab profiling results
LAST_RESULTS = None

_BLANK_TABLES = ("exp_and_others", "natural_log", "exp_and_friends")
_tables_patched = False


def _patch_act_tables():
    """Steer the ACT-table-load chooser toward natural_log_exp_and_others so
    exp+ln never alternate table loads. Only the (name -> funcs) map used by
    the load-insertion pass and CoreSim is filtered; emitted act_func_set_ids
    still index the real act_info.json, so walrus/hardware see valid sets."""
    global _tables_patched
    if _tables_patched:
        return
    _tables_patched = True
    from concourse import hw_specs, bass_interp
    orig = hw_specs.get_activation_tables

    def patched(arch):
        tabs = dict(orig(arch))
        for name in _BLANK_TABLES:
            if name in tabs:
                tabs[name] = set()
        return tabs

    bacc.get_activation_tables = patched
    bass_interp.get_activation_tables = patched



def _emit_kernel(ctx: ExitStack, tc: "tile.TileContext", ap: dict):
    nc = tc.nc

    # Chain all PE instructions in program order (PE executes in-order; this
    # keeps the Tile scheduler from reordering matmuls within a PSUM bank,
    # which would break has_written clear ordering).
    pe_prev = [None]

    def mm(*args, **kw):
        inst = nc.tensor.matmul(*args, **kw)
        if kw.get("skip_group_check") or kw.get("start") in (False, None):
            # keep explicit order only for matmuls that join open psum groups
            if pe_prev[0] is not None:
                add_dep_helper(inst.ins, pe_prev[0], sync=False,
                               reason="pe-order")
        pe_prev[0] = inst.ins
        return inst

    def tr_(out, in_, ident):
        inst = nc.tensor.transpose(out, in_, ident)
        if pe_prev[0] is not None:
            add_dep_helper(inst.ins, pe_prev[0], sync=False, reason="pe-order")
        pe_prev[0] = inst.ins
        return inst

    xT, wb1d, wb0d, woutT = ap["xT"], ap["wb1"], ap["wb0"], ap["woutT"]
    bgk2, cb = ap["bgk2"], ap["cb"]
    out = ap["out"]

    consts = ctx.enter_context(tc.tile_pool(name="consts", bufs=1))
    wpool = ctx.enter_context(tc.tile_pool(name="wpool", bufs=1))
    work = ctx.enter_context(tc.tile_pool(name="work", bufs=3))
    wst = ctx.enter_context(tc.tile_pool(name="wst", bufs=2))
    outp = ctx.enter_context(tc.tile_pool(name="outp", bufs=3))
    ppool = ctx.enter_context(tc.tile_pool(name="ppool", bufs=2, space="PSUM"))
    ptr = ctx.enter_context(tc.tile_pool(name="ptr", bufs=2, space="PSUM"))
    pbf = ctx.enter_context(tc.tile_pool(name="pbf", bufs=1, space="PSUM"))
    pao = ctx.enter_context(tc.tile_pool(name="pao", bufs=1, space="PSUM"))

    # PE clock warmup source: FIRST vector instruction so the warmup matmuls
    # can launch while the DMA rings fill. The tensor engine needs ~3us of
    # continuous execution to reach max frequency.
    warm = consts.tile([128, 512], BF16)
    nc.vector.memset(warm[:], 0.0)

    # ---- DMA schedule: need-ordered across the three ~110GB/s rings.
    # Ring layout (issue order = per-engine program order):
    #   gpsimd: x0 | cb(Lb|idb) | x1 | x2 | x3 | x6
    #   sync:   wb1[0:4] | wb0[0:2] | wb0[4:6] | wout0 | x4
    #   scalar: bgk2 | wb1[4:8] | wb0[2:4] | wb0[6:8] | wout1 | x5 | x7
    # bank1 (gate|z) + x0 land ~10.5us -> P1(0) starts; wb0 quarters pace
    # P0(0) to ~15.5us; later x chunks have chunk-pipeline slack.
    xsb = wpool.tile([128, N, NK], BF16)
    wb1_sb = wpool.tile([128, NK, 384], BF16)
    wb0_sb = wpool.tile([128, NK, 512], BF16)
    wout_sb = wpool.tile([128, 2, D], BF16)
    cb_sb = consts.tile([128, 256], BF16)
    bg_sb = consts.tile([1, 128], BF16)

    nc.gpsimd.dma_start(out=xsb[:, 0:C, :], in_=xT[:, 0:C, :])
    nc.scalar.dma_start(out=bg_sb[:], in_=bgk2[:])
    nc.sync.dma_start(out=wb1_sb[:, 0:2, :], in_=wb1d[:, 0:2, :])
    nc.scalar.dma_start(out=wb1_sb[:, 2:4, :], in_=wb1d[:, 2:4, :])
    nc.sync.dma_start(out=wb1_sb[:, 4:6, :], in_=wb1d[:, 4:6, :])
    nc.scalar.dma_start(out=wb1_sb[:, 6:8, :], in_=wb1d[:, 6:8, :])
    nc.gpsimd.dma_start(out=cb_sb[:], in_=cb[:])
    nc.sync.dma_start(out=wb0_sb[:, 0:2, :], in_=wb0d[:, 0:2, :])
    nc.scalar.dma_start(out=wb0_sb[:, 2:4, :], in_=wb0d[:, 2:4, :])
    nc.gpsimd.dma_start(out=xsb[:, C:2 * C, :], in_=xT[:, C:2 * C, :])
    nc.sync.dma_start(out=wb0_sb[:, 4:6, :], in_=wb0d[:, 4:6, :])
    nc.scalar.dma_start(out=wb0_sb[:, 6:8, :], in_=wb0d[:, 6:8, :])
    nc.gpsimd.dma_start(out=xsb[:, 2 * C:3 * C, :], in_=xT[:, 2 * C:3 * C, :])
    nc.sync.dma_start(out=wout_sb[:, 0, :], in_=woutT[0])
    nc.scalar.dma_start(out=wout_sb[:, 1, :], in_=woutT[1])
    nc.gpsimd.dma_start(out=xsb[:, 3 * C:4 * C, :], in_=xT[:, 3 * C:4 * C, :])
    nc.sync.dma_start(out=xsb[:, 4 * C:5 * C, :], in_=xT[:, 4 * C:5 * C, :])
    nc.scalar.dma_start(out=xsb[:, 5 * C:6 * C, :], in_=xT[:, 5 * C:6 * C, :])
    nc.gpsimd.dma_start(out=xsb[:, 6 * C:7 * C, :], in_=xT[:, 6 * C:7 * C, :])
    nc.scalar.dma_start(out=xsb[:, 7 * C:8 * C, :], in_=xT[:, 7 * C:8 * C, :])

    # remaining consts on vector (idle during the load phase)
    ones_row = consts.tile([1, 128], BF16)
    nc.vector.memset(ones_row[:], 1.0)
    ones_col = consts.tile([128, 1], BF16)
    nc.vector.memset(ones_col[:], 1.0)
    eps_sb = consts.tile([128, 1], F32)
    nc.vector.memset(eps_sb[:], EPS)
    Lb_sb = cb_sb[:, 0:128]               # L[s,t]=1 iff s<=t (triu), bf16
    idb_sb = cb_sb[:, 128:256]            # identity, bf16
    L_sb = consts.tile([128, 128], F32)
    nc.vector.tensor_copy(L_sb[:], cb_sb[:, 0:128])

    # PE clock warmup: dummy matmuls while the first DMAs land.
    wps = pao.tile([128, 512], F32, tag="big")
    for i in range(8):
        mm(wps[:], lhsT=warm[:, 0:128], rhs=warm[:],
           start=(i == 0), stop=(i == 7))

    # ---- main loop ---------------------------------------------------------
    # proj psum [128,1024]: bank0 {q 0:128 | k 128:256 | v 256:512}
    # bank1 {gate 512:768 | z 768:896 | b_loc 896:1024}. bank1 (and its bias
    # close) is emitted BEFORE bank0 so softplus overlaps the qkv matmuls.
    # b (token-major) and bT (feature-major) are both produced directly by
    # matmuls against the triangular mask (b = L^T g, bT = g^T L).
    # The ENTIRE output path (silu gate via reciprocal_approx_fast - no
    # activation-table switch - RMS scale, final projection, store) is inlined
    # per chunk, one chunk behind the front of the pipeline, so outputs
    # stream to HBM throughout the loop and no drain phase remains.

    def P1(c):
        proj = ppool.tile([128, 1024], F32, tag="proj")
        tok = slice(c * C, (c + 1) * C)
        for k in range(NK):
            mm(proj[:, 512:896], lhsT=xsb[:, tok, k], rhs=wb1_sb[:, k, :],
               start=(k == 0), stop=False)
        bias_mm = mm(proj[:, 768:896], lhsT=ones_row[:], rhs=bg_sb[:],
                     start=False, stop=True)
        # softplus part a: e1 = exp(-z)
        e1 = work.tile([128, 128], F32, tag="e1")
        i = nc.scalar.activation(e1[:], proj[:, 768:896], AF.Exp, scale=-1.0)
        add_dep_helper(i.ins, bias_mm.ins, sync=False, reason="z after close")
        return proj, e1

    def SPb(c, e1):
        u1 = work.tile([128, 128], F32, tag="u1")
        nc.vector.tensor_scalar(u1[:], e1[:], 1.0, E48, ALU.add, ALU.min)
        return u1

    def SPc(c, u1):
        g_c = work.tile([128, 128], BF16, tag="g")
        nc.scalar.activation(g_c[:], u1[:], AF.Ln)
        return g_c

    def P0(c, proj):
        tok = slice(c * C, (c + 1) * C)
        for k in range(NK):
            mm(proj[:, 0:512], lhsT=xsb[:, tok, k], rhs=wb0_sb[:, k, :],
               start=(k == 0), stop=(k == NK - 1))

    def Bmm(c, proj, g_c):
        bmm = mm(proj[:, 896:1024], lhsT=Lb_sb, rhs=g_c[:],
                 start=False, stop=False, skip_group_check=True)
        sml = ptr.tile([128, 512], F32, tag="sml")   # bT | at | ssq
        mm(sml[:, 0:128], lhsT=g_c[:], rhs=Lb_sb, start=True, stop=True)
        return sml, bmm

    def Ex(c, proj, sml, bmm):
        En_tok = work.tile([128, 128], BF16, tag="Ent")
        i = nc.scalar.activation(En_tok[:], proj[:, 896:1024], AF.Exp,
                                 scale=1.0 / GLN)
        add_dep_helper(i.ins, bmm.ins, sync=False, reason="b after b-mm")
        ET = work.tile([128, 128], BF16, tag="ET")
        nc.scalar.activation(ET[:], sml[:, 0:128], AF.Exp, scale=-1.0 / GLN)
        EnT = work.tile([128, 128], BF16, tag="EnT")
        nc.scalar.activation(EnT[:], sml[:, 0:128], AF.Exp, scale=1.0 / GLN)
        f_vec = work.tile([128, 1], F32, tag="f")
        nc.scalar.activation(f_vec[:], sml[:, 127:128], AF.Exp, scale=-1.0 / GLN)
        # silu ingredient: eg = exp(-ug) straight from psum
        eg = work.tile([128, DV], F32, tag="eg")
        nc.scalar.activation(eg[:], proj[:, 512:768], AF.Exp, scale=-1.0)
        return En_tok, ET, EnT, f_vec, eg

    def QK(c, proj):
        qk_sb = work.tile([128, 256], BF16, tag="qk")
        nc.vector.tensor_copy(qk_sb[:], proj[:, 0:256])
        v_tm = work.tile([128, DV], BF16, tag="v")
        # vector, not scalar: the ACT queue is the pole since it took the
        # second o_sb cast, and a late SPc/g_c stalls the next chunk's Bmm
        nc.vector.tensor_copy(v_tm[:], proj[:, 256:512])
        ug = work.tile([128, DV], F32, tag="ug")
        nc.scalar.copy(ug[:], proj[:, 512:768])
        return qk_sb, v_tm, ug

    def T(c, qk_sb):
        # one psum bank holds this iteration's 4 transposes: qT|kT|gateT(c-1)
        tq = pbf.tile([128, 512], BF16, tag="tqk")
        tr_(tq[:, 0:128], qk_sb[:, 0:128], idb_sb)
        tr_(tq[:, 128:256], qk_sb[:, 128:256], idb_sb)
        return tq

    def M(c, tq, qk_sb, En_tok, ET, EnT):
        qtT = work.tile([128, 128], BF16, tag="qtT")
        nc.vector.tensor_mul(qtT[:], tq[:, 0:128], ET[:])
        ktT = work.tile([128, 128], BF16, tag="ktT")
        nc.vector.tensor_mul(ktT[:], tq[:, 128:256], EnT[:])
        kt_tm = work.tile([128, 128], BF16, tag="kt")
        nc.vector.tensor_mul(kt_tm[:], qk_sb[:, 128:256], En_tok[:])
        return qtT, ktT, kt_tm

    def SILU(c, eg):
        # silu: rf = 1/(1+eg); emitted after at_m so the critical DVE ops
        # (qtT/ktT/kt_tm/og/at) run first. Pool engine is NOT used for
        # per-chunk ops: its dispatch latency is multi-us.
        dg = work.tile([128, DV], F32, tag="dg")
        nc.vector.tensor_scalar_add(dg[:], eg[:], 1.0)
        rf = work.tile([128, DV], F32, tag="rf")
        nc.vector.reciprocal_approx_fast(rf[:], dg[:])
        return rf

    def A(c, sml, qtT, ktT):
        mm(sml[:, 128:256], lhsT=ktT[:], rhs=qtT[:], start=True, stop=True)

    def AM(c, sml):
        at_m = work.tile([128, 128], BF16, tag="atm")
        nc.vector.tensor_mul(at_m[:], sml[:, 128:256], L_sb[:])
        return at_m

    def OT(c, at_m, qtT, v_tm):
        big = pao.tile([128, 512], F32, tag="big")
        ot = big[:, 0:256]
        if c > 0:
            w_prev = state["w_prev_for_o"]
            mm(ot[:, 0:128], lhsT=w_prev[:, 0:128], rhs=qtT[:],
               start=True, stop=False)
            mm(ot[:, 128:256], lhsT=w_prev[:, 128:256], rhs=qtT[:],
               start=False, stop=False, skip_group_check=True)
            mm(ot[:, 0:128], lhsT=v_tm[:, 0:128], rhs=at_m[:],
               start=False, stop=False, skip_group_check=True)
        else:
            mm(ot[:, 0:128], lhsT=v_tm[:, 0:128], rhs=at_m[:],
               start=True, stop=False)
        mm(ot[:, 128:256], lhsT=v_tm[:, 128:256], rhs=at_m[:],
           start=False, stop=False, skip_group_check=True)
        return big

    def ST(c, big, kt_tm, v_tm, f_vec):
        if c == NCH - 1:
            return   # final state is never consumed
        st = big[:, 256:512]
        mm(st[:], lhsT=kt_tm[:], rhs=v_tm[:], start=True, stop=False,
           skip_group_check=True)
        if c > 0:
            mm(st[:], lhsT=idb_sb, rhs=state["w_prev"][:], start=False,
               stop=False, skip_group_check=True)
        w_new = wst.tile([128, DV], BF16, tag="w")
        nc.vector.tensor_scalar(w_new[:], st[:], f_vec[:], None, ALU.mult)
        state["w_prev"] = w_new

    def SQ(c, big):
        sq = work.tile([128, DV], BF16, tag="sq")
        nc.scalar.square(sq[:], big[:, 0:256])
        return sq

    def SSQ(c, sq, sml):
        ssq = sml[:, 256:257]
        mm(ssq, lhsT=sq[:, 0:128], rhs=ones_col[:],
           start=True, stop=False, skip_group_check=True)
        mm(ssq, lhsT=sq[:, 128:256], rhs=ones_col[:],
           start=False, stop=False, skip_group_check=True)
        return ssq

    def R(c, ssq):
        s_c = work.tile([128, 1], F32, tag="s")
        nc.scalar.activation(s_c[:], ssq, AF.Ln, scale=1.0 / DV, bias=eps_sb[:])
        r_c = work.tile([128, 1], F32, tag="r")
        nc.scalar.activation(r_c[:], s_c[:], AF.Exp, scale=-0.5)
        return r_c

    def GATE(c, ug, rf, r_c):
        # gate*r = (ug*r) * sigmoid(ug), sigmoid via fast reciprocal
        gate_tm = work.tile([128, DV], BF16, tag="gate")
        nc.vector.scalar_tensor_tensor(gate_tm[:], ug[:], r_c[:], rf[:],
                                       ALU.mult, ALU.mult)
        return gate_tm

    def TR2(c, gate_tm, tq):
        tr2 = tq[:, 256:512]
        tr_(tr2[:, 0:128], gate_tm[:, 0:128], idb_sb)
        tr_(tr2[:, 128:256], gate_tm[:, 128:256], idb_sb)
        return tr2

    def OG(c, tr2, big):
        # gateT eviction on ACT: it has ~2us of slack before og needs it,
        # and the DVE is the busier queue in steady state
        gateT = work.tile([128, DV], F32, tag="gT")
        nc.scalar.copy(gateT[:], tr2[:])
        og = work.tile([128, DV], BF16, tag="og")
        nc.vector.tensor_mul(og[:], big[:, 0:256], gateT[:])
        return og

    def FINmm(c, og):
        fin = ppool.tile([128, 1024], F32, tag="proj")
        for nb in range(2):
            cols = slice(nb * 512, (nb + 1) * 512)
            mm(fin[:, cols], lhsT=og[:, 0:128],
               rhs=wout_sb[:, 0, cols], start=True, stop=False)
            mm(fin[:, cols], lhsT=og[:, 128:256],
               rhs=wout_sb[:, 1, cols], start=False, stop=True)
        return fin

    def FINout(c, fin):
        # emitted late so w_new precedes the casts in the vector queue
        tok0 = c * C
        o_sb = outp.tile([128, 1024], BF16, tag="o")
        if c == NCH - 1:
            # parallel casts; each column half stores as soon as its own
            # cast lands (don't gate the first store on both casts)
            nc.vector.tensor_copy(o_sb[:, 0:512], fin[:, 0:512])
            nc.sync.dma_start(out=out[tok0:tok0 + C, 0:512],
                              in_=o_sb[:, 0:512])
            nc.scalar.copy(o_sb[:, 512:1024], fin[:, 512:1024])
            nc.scalar.dma_start(out=out[tok0:tok0 + C, 512:1024],
                                in_=o_sb[:, 512:1024])
        else:
            # parallel halves: two serial vector casts end ~0.7us past the
            # chunk period and stall P1(c+2) on the ppool WAR
            nc.vector.tensor_copy(o_sb[:, 0:512], fin[:, 0:512])
            nc.scalar.copy(o_sb[:, 512:1024], fin[:, 512:1024])
            nc.sync.dma_start(out=out[tok0:tok0 + C, :], in_=o_sb[:])

    # ---- pipeline driver ----
    state = {"w_prev": None, "w_prev_for_o": None}
    pend = {}
    prv = None   # chunk c-1's (ug, rf, big, r) for the interleaved output tail

    # Prologue: project chunks 0 AND 1's bank1 while wb0/x stream in — the
    # PE would otherwise idle ~3us waiting for the q|k|v weight quarters.
    # The loop then emits P1(c+1) only from iteration 1 on, preserving the
    # proj/fin ppool rotation (depth stays 2 - no forward-wait deadlock).
    proj0, e1_0 = P1(0)
    u1_0 = SPb(0, e1_0)
    g_0 = SPc(0, u1_0)
    proj1, e1_1 = P1(1)
    u1_1 = SPb(1, e1_1)
    g_1 = SPc(1, u1_1)
    P0(0, proj0)
    qk0, v0, ug0 = QK(0, proj0)
    pend[0] = dict(proj=proj0, g=g_0, qk=qk0, v=v0, ug=ug0)
    pend[1] = dict(proj=proj1, g=g_1)

    for c in range(NCH):
        p = pend[c]
        proj, g_c = p["proj"], p["g"]
        if prv is not None:
            r_p = R(c - 1, prv["ssq"])
        sml, bmm = Bmm(c, proj, g_c)
        En_tok, ET, EnT, f_vec, eg = Ex(c, proj, sml, bmm)
        # qk/v/ug were evicted LAST iteration (right after P0(c) closed):
        # T(c) below never waits on the DVE for the qk cast
        qk_sb, v_tm, ug = p["qk"], p["v"], p["ug"]
        if prv is not None:
            gate_p = GATE(c - 1, prv["ug"], prv["rf"], r_p)
        # transposes run before the c+1 projections on the in-order PE so
        # the DVE products (qtT/ktT/kt_tm/og) are all ready long before A
        tq = T(c, qk_sb)
        if prv is not None:
            tr2_p = TR2(c - 1, gate_p, tq)
        qtT, ktT, kt_tm = M(c, tq, qk_sb, En_tok, ET, EnT)
        if prv is not None:
            og_p = OG(c - 1, tr2_p, prv["big"])
        if c + 1 < NCH:
            if (c + 1) not in pend:
                projn, e1n = P1(c + 1)
                u1n = SPb(c + 1, e1n)
                P0(c + 1, projn)
                gn = SPc(c + 1, u1n)
                pend[c + 1] = dict(proj=projn, g=gn)
            else:
                projn = pend[c + 1]["proj"]
                P0(c + 1, projn)
            qkn, vn, ugn = QK(c + 1, projn)
            pend[c + 1].update(qk=qkn, v=vn, ug=ugn)
        A(c, sml, qtT, ktT)
        at_m = AM(c, sml)
        rf = SILU(c, eg)
        state["w_prev_for_o"] = state["w_prev"]
        # OT before FINmm: OT's at_m is ready right after A, while FIN's og
        # arrives later - this order gives the og chain ~0.7us more slack
        big = OT(c, at_m, qtT, v_tm)
        if prv is not None:
            fin_p = FINmm(c - 1, og_p)
        sq = SQ(c, big)
        ST(c, big, kt_tm, v_tm, f_vec)
        ssq_c = SSQ(c, sq, sml)
        if prv is not None:
            FINout(c - 1, fin_p)
        prv = dict(ug=ug, rf=rf, big=big, ssq=ssq_c)

    # last chunk's output tail
    r_p = R(NCH - 1, prv["ssq"])
    gate_p = GATE(NCH - 1, prv["ug"], prv["rf"], r_p)
    t_last = pbf.tile([128, 512], BF16, tag="tqk")
    tr2_p = TR2(NCH - 1, gate_p, t_last)
    og_p = OG(NCH - 1, tr2_p, prv["big"])
    fin_l = FINmm(NCH - 1, og_p)
    FINout(NCH - 1, fin_l)

def _build_nc():
    _patch_act_tables()
    nc = bacc.Bacc("TRN2", target_bir_lowering=False, debug=False, num_devices=8)
    ap = {
        "xT": nc.dram_tensor("xT", [128, N, NK], BF16, kind="ExternalInput").ap(),
        "wb1": nc.dram_tensor("wb1", [128, NK, 384], BF16,
                              kind="ExternalInput").ap(),
        "wb0": nc.dram_tensor("wb0", [128, NK, 512], BF16,
                              kind="ExternalInput").ap(),
        "woutT": nc.dram_tensor("woutT", [2, 128, D], BF16,
                                kind="ExternalInput").ap(),
        "bgk2": nc.dram_tensor("bgk2", [1, 128], BF16, kind="ExternalInput").ap(),
        "cb": nc.dram_tensor("cb", [128, 256], BF16, kind="ExternalInput").ap(),
        "out": nc.dram_tensor("out", [N, D], BF16, kind="ExternalOutput").ap(),
    }
    with tile.TileContext(nc) as tc:
        with ExitStack() as ctx:
            _emit_kernel(ctx, tc, ap)
    nc.compile()
    return nc


def kernel(x, Wq, Wk, Wv, Wg, Wgk1, Wgk2, bgk2, Wout, rms_w):
    global LAST_RESULTS
    BF = ml_dtypes.bfloat16
    x = np.asarray(x, np.float32)
    Wz = (np.asarray(Wgk1, np.float32) @ np.asarray(Wgk2, np.float32))
    L = np.triu(np.ones((C, C), np.float32))
    I32 = np.eye(128, dtype=np.float32)
    cb = np.ascontiguousarray(
        np.concatenate([L, I32], axis=1)).astype(BF)

    in_maps = []
    for core in range(8):
        b, h = core // H, core % H
        xTb = np.ascontiguousarray(
            x[b].T.reshape(NK, 128, N).transpose(1, 2, 0)).astype(BF)
        blob = np.concatenate([
            Wq[:, h * DK:(h + 1) * DK], Wk[:, h * DK:(h + 1) * DK],
            Wv[:, h * DV:(h + 1) * DV], Wg[:, h * DV:(h + 1) * DV],
            Wz[:, h * DK:(h + 1) * DK]],
            axis=1).astype(np.float32).reshape(NK, 128, 896)
        wb0 = np.ascontiguousarray(
            blob[:, :, 0:512].transpose(1, 0, 2)).astype(BF)
        wb1 = np.ascontiguousarray(
            blob[:, :, 512:896].transpose(1, 0, 2)).astype(BF)
        woutP = np.ascontiguousarray(
            (np.asarray(rms_w, np.float32)[:, None]
             * np.asarray(Wout, np.float32)[h * DV:(h + 1) * DV])
        ).reshape(2, 128, D).astype(BF)
        in_maps.append({
            "xT": xTb,
            "wb1": wb1,
            "wb0": wb0,
            "woutT": woutP,
            "bgk2": np.ascontiguousarray(
                np.asarray(bgk2, np.float32)[h * DK:(h + 1) * DK][None, :]
            ).astype(BF),
            "cb": cb,
        })

    nc = _build_nc()
    trace = os.environ.get("BASSGLA_TRACE", "0") == "1"
    res = run_bass_kernel_spmd(nc, in_maps, list(range(8)), trace=trace)
    LAST_RESULTS = res

    out = np.zeros((B, N, D), np.float32)
    for core in range(8):
        out[core // H] += np.asarray(res.results[core]["out"], np.float32)
    return out
